# revision 12
# baseline (speedup 1.0000x reference)
"""AdaptConv2d Trainium2 kernel: 8-core data-parallel, gate-driven sparse conv.

Computes, per sample b:
  layer_bit = (LSTM-gate pre-activation > 0)
  if layer_bit:  channel mask m_c = (channel-gate fc pre-activation > 0)
                 out[c] = conv3x3(x)[c] if m_c else x[c]
  else:          out = x

Device strategy per core (4 samples):
  - x loaded into zero-padded (58x58) SBUF images, f32r (tf32) typed for the
    TensorEngine fast path; bits are untouched so pass-through output is exact.
  - Layer gate (GAP + 1x1-conv + single-step LSTM + fc) in true fp32 for all 4
    samples, branch-free.
  - Per sample, a 0/1-trip For_i (trip = layer bit) guards the heavy work:
    stride-2 channel-gate conv (tf32 matmuls, fp32 accum), fp32 fc -> binary
    mask, prefix-sum -> one-hot selection matrix S, PE-side weight gather
    (W^T @ S), compact conv over only ceil(n_active/128) 128-channel blocks
    (inner 0/1-trip For_i for the second block), and an indirect row-scatter
    of conv rows into the output (out-of-bounds pad rows silently dropped).
  - Unconditional default write out = x covers inactive channels/samples.
"""

import os
import sys
import types

sys.path.insert(0, "/opt/trn_rl_repo")

import numpy as np

# antenv.axon_hooks is missing from this image; inject a minimal stand-in so
# run_bass_kernel_spmd's trace path imports cleanly (used only when tracing).
try:
    import antenv  # noqa: F401

    if "antenv.axon_hooks" not in sys.modules:
        _m = types.ModuleType("antenv.axon_hooks")
        _h = [None]
        _m.set_axon_ntff_profile_hook = lambda hook: _h.__setitem__(0, hook)
        _m.get_axon_ntff_profile_hook = lambda: _h[0]
        sys.modules["antenv.axon_hooks"] = _m
        antenv.axon_hooks = _m
except Exception:
    pass

import concourse.bass as bass
import concourse.mybir as mybir
from concourse import bacc
from concourse.expressions import smin
from concourse.tile import TileContext
from concourse.bass_utils import run_bass_kernel_spmd

F32 = mybir.dt.float32
F32R = mybir.dt.float32r
I32 = mybir.dt.int32
AF = mybir.ActivationFunctionType
ALU = mybir.AluOpType

B, C, H, W = 32, 256, 56, 56
NCORES = 8
BS = B // NCORES          # samples per core
HW = H * W                # 3136
PH, PW = H + 2, W + 2     # 58x58 padded image
PHW = PH * PW             # 3364
XT_COLS = PHW + 4         # tail pad: edge-tap reads run 2 past the image
LSTM_H = 10
ENGINES = list(mybir.ALL_ENGINES)

# main-conv spatial chunking: 7 chunks x 8 valid rows; each chunk is a
# contiguous 464-wide span of the padded image (includes L/R pad cols, whose
# outputs are junk and excluded at extraction time)
NCHUNK = 7
CH_ROWS = 8
CH_N = CH_ROWS * PH       # 464

# channel-gate conv: 27x27 valid outputs, row-chunks of 14/13, 28 cols (28th
# col junk so the fp32r moving operand has an even innermost count)
G_CHUNKS = ((0, 14), (14, 27))
G_COLS = 28

_CACHE = {}


def _build():
    nc = bacc.Bacc(None, target_bir_lowering=False)

    xp = nc.declare_dram_parameter("x", [BS, C, H, W], F32R, isOutput=False)
    outp = nc.declare_dram_parameter("out", [BS, C, HW], F32R, isOutput=True)
    wnat = nc.declare_dram_parameter("wnat", [2, 128, 9 * C], F32R, isOutput=False)
    cgw = nc.declare_dram_parameter("cgw", [2, 128, 9 * C], F32R, isOutput=False)
    fcwt = nc.declare_dram_parameter("fcwt", [2, 128, C], F32, isOutput=False)
    lgwt = nc.declare_dram_parameter("lgwt", [2, 128, LSTM_H], F32, isOutput=False)
    wiht = nc.declare_dram_parameter("wiht", [LSTM_H + 1, 4 * LSTM_H], F32, isOutput=False)
    lgfc = nc.declare_dram_parameter("lgfc", [1, LSTM_H], F32, isOutput=False)
    cgb = nc.declare_dram_parameter("cgb", [128, 2], F32, isOutput=False)
    fcb = nc.declare_dram_parameter("fcb", [128, 2], F32, isOutput=False)
    lgb = nc.declare_dram_parameter("lgb", [LSTM_H, 1], F32, isOutput=False)
    lfb = nc.declare_dram_parameter("lfb", [1, 1], F32, isOutput=False)
    ucon = nc.declare_dram_parameter("ucon", [128, 128], F32, isOutput=False)
    onesk = nc.declare_dram_parameter("onesk", [128, 128], F32, isOutput=False)
    jcon = nc.declare_dram_parameter("jcon", [128, 2 * 128], F32, isOutput=False)
    cvec = nc.declare_dram_parameter("cvec", [128, 2], F32, isOutput=False)
    dbg = nc.declare_dram_parameter("dbg", [128, 16], F32, isOutput=True)

    with TileContext(nc) as tc:
        with tc.tile_pool(name="sbuf", bufs=1) as pc, \
             tc.tile_pool(name="work", bufs=1) as pw, \
             tc.tile_pool(name="psum", bufs=1, space="PSUM") as pp:

            # ---- constants / weights resident in SBUF ----
            ucon_t = pc.tile([128, 128], F32, tag="ucon")
            nc.sync.dma_start(out=ucon_t[:], in_=ucon[:])
            ones_t = pc.tile([128, 128], F32, tag="ones")
            nc.sync.dma_start(out=ones_t[:], in_=onesk[:])
            j_t = pc.tile([128, 256], F32, tag="jcon")
            nc.sync.dma_start(out=j_t[:], in_=jcon[:])
            cvec_t = pc.tile([128, 2], F32, tag="cvec")
            nc.sync.dma_start(out=cvec_t[:], in_=cvec[:])
            lgwt_t = pc.tile([128, 2 * LSTM_H], F32, tag="lgwt")
            nc.sync.dma_start(out=lgwt_t[:, 0:LSTM_H], in_=lgwt[0])
            nc.sync.dma_start(out=lgwt_t[:, LSTM_H:2 * LSTM_H], in_=lgwt[1])
            wiht_t = pc.tile([LSTM_H + 1, 4 * LSTM_H], F32, tag="wiht")
            nc.sync.dma_start(out=wiht_t[:], in_=wiht[:])
            lgb_t = pc.tile([LSTM_H, 1], F32, tag="lgb")
            nc.sync.dma_start(out=lgb_t[:], in_=lgb[:])
            lgfc_t = pc.tile([1, LSTM_H], F32, tag="lgfc")
            nc.sync.dma_start(out=lgfc_t[:], in_=lgfc[:])
            lfb_t = pc.tile([1, 1], F32, tag="lfb")
            nc.sync.dma_start(out=lfb_t[:], in_=lfb[:])

            # broadcast tiny layer-gate constants across the 4 sample rows
            lrow_b = pc.tile([BS, LSTM_H], F32, tag="lrowb")
            nc.gpsimd.partition_broadcast(lrow_b[:], lgfc_t[:])
            fb_b = pc.tile([BS, 1], F32, tag="fbb")
            nc.gpsimd.partition_broadcast(fb_b[:], lfb_t[:])
            zeros4 = pc.tile([BS, 1], F32, tag="z4")
            nc.vector.memset(zeros4[:], 0.0)
            zeros128 = pc.tile([128, 1], F32, tag="z128")
            nc.vector.memset(zeros128[:], 0.0)

            # ---- x into padded SBUF images + default pass-through writes ----
            xts = []
            for si in range(BS):
                for kb in range(2):
                    xt = pc.tile([128, XT_COLS], F32R, tag=f"x_{si}_{kb}")
                    xv = xt[:, 0:PHW].rearrange("p (h w) -> p h w", h=PH)
                    nc.vector.memset(xv[:, 0:1, :].bitcast(F32), 0.0)
                    nc.vector.memset(xv[:, PH - 1:PH, :].bitcast(F32), 0.0)
                    nc.vector.memset(xv[:, :, 0:1].bitcast(F32), 0.0)
                    nc.vector.memset(xv[:, :, PW - 1:PW].bitcast(F32), 0.0)
                    nc.vector.memset(xt[:, PHW:XT_COLS].bitcast(F32), 0.0)
                    nc.sync.dma_start(out=xv[:, 1:PH - 1, 1:PW - 1],
                                      in_=xp[si, kb * 128:(kb + 1) * 128])
                    nc.sync.dma_start(
                        out=outp[si, kb * 128:(kb + 1) * 128].rearrange(
                            "p (h w) -> p h w", h=H),
                        in_=xv[:, 1:PH - 1, 1:PW - 1])
                    xts.append(xt)

            def xtile(si, kb):
                return xts[si * 2 + kb]

            # ---- layer gate for all samples (true fp32, branch-free) ----
            g1 = pc.tile([128, 2 * BS], F32, tag="g1")  # GAP sums, col = kb*BS+si
            for si in range(BS):
                for kb in range(2):
                    xv = xtile(si, kb)[:, 0:PHW].bitcast(F32).rearrange(
                        "p (h w) -> p h w", h=PH)
                    nc.vector.tensor_reduce(
                        out=g1[:, kb * BS + si:kb * BS + si + 1],
                        in_=xv[:, 1:PH - 1, 1:PW - 1],
                        axis=mybir.AxisListType.XY, op=ALU.add)

            ph = pp.tile([LSTM_H, BS], F32, tag="small")
            for kb in range(2):
                nc.tensor.matmul(ph[:], lgwt_t[:, kb * LSTM_H:(kb + 1) * LSTM_H],
                                 g1[:, kb * BS:(kb + 1) * BS],
                                 start=(kb == 0), stop=(kb == 1))
            htile = pc.tile([LSTM_H + 1, BS], F32, tag="htile")
            nc.sync.dma_start(out=htile[LSTM_H:LSTM_H + 1, :],
                              in_=onesk[0:1, 0:BS])
            nc.scalar.activation(htile[0:LSTM_H, :], ph[:], AF.Relu,
                                 bias=lgb_t[:, 0:1], scale=1.0 / HW)
            pg = pp.tile([BS, 4 * LSTM_H], F32, tag="small")
            nc.tensor.matmul(pg[:], htile[:], wiht_t[:], start=True, stop=True)
            lwork = pc.tile([BS, 4 * LSTM_H], F32, tag="lwork")
            nc.scalar.activation(lwork[:, 0:LSTM_H], pg[:, 0:LSTM_H], AF.Sigmoid)
            nc.scalar.activation(lwork[:, 3 * LSTM_H:4 * LSTM_H],
                                 pg[:, 3 * LSTM_H:4 * LSTM_H], AF.Sigmoid)
            nc.scalar.activation(lwork[:, 2 * LSTM_H:3 * LSTM_H],
                                 pg[:, 2 * LSTM_H:3 * LSTM_H], AF.Tanh)
            cbuf = pc.tile([BS, LSTM_H], F32, tag="cbuf")
            nc.vector.tensor_tensor(out=cbuf[:], in0=lwork[:, 0:LSTM_H],
                                    in1=lwork[:, 2 * LSTM_H:3 * LSTM_H],
                                    op=ALU.mult)
            ebuf = pc.tile([BS, LSTM_H], F32, tag="ebuf")
            nc.scalar.activation(ebuf[:], cbuf[:], AF.Tanh)
            hsb = pc.tile([BS, LSTM_H], F32, tag="hsb")
            nc.vector.tensor_tensor(out=hsb[:], in0=lwork[:, 3 * LSTM_H:4 * LSTM_H],
                                    in1=ebuf[:], op=ALU.mult)
            prod = pc.tile([BS, LSTM_H], F32, tag="prod")
            nc.vector.tensor_tensor(out=prod[:], in0=hsb[:], in1=lrow_b[:],
                                    op=ALU.mult)
            lpre = pc.tile([BS, 1], F32, tag="lpre")
            nc.vector.tensor_reduce(out=lpre[:], in_=prod[:],
                                    axis=mybir.AxisListType.X, op=ALU.add)
            l_bin = pc.tile([BS, 1], F32, tag="lbin")
            nc.vector.scalar_tensor_tensor(out=l_bin[:], in0=lpre[:],
                                           scalar=fb_b[:, 0:1], in1=zeros4[:],
                                           op0=ALU.add, op1=ALU.is_gt)
            dbg_t = pc.tile([128, 16], F32, tag="dbg")
            nc.vector.memset(dbg_t[:], 0.0)
            nc.vector.tensor_copy(out=dbg_t[0:BS, 0:1], in_=lpre[:])
            nc.vector.tensor_copy(out=dbg_t[0:BS, 1:2], in_=l_bin[:])
            nc.vector.tensor_copy(out=dbg_t[0:128, 2:4], in_=g1[:, 0:2 * BS:4])
            l_i32 = pc.tile([BS, 1], I32, tag="li32")
            nc.vector.tensor_copy(out=l_i32[:], in_=l_bin[:])

            l_vals = [nc.values_load(l_i32[si:si + 1, 0:1], engines=ENGINES,
                                     min_val=0, max_val=1,
                                     skip_runtime_bounds_check=True)
                      for si in range(BS)]

            # gate/conv weights only needed if any sample is layer-on
            any_on = l_vals[0] + l_vals[1] + l_vals[2] + l_vals[3]
            wnat_t = pc.tile([128, 2 * 9 * C], F32R, tag="wnat")
            cgw_t = pc.tile([128, 2 * 9 * C], F32R, tag="cgw")
            fcwt_t = pc.tile([128, 2 * C], F32, tag="fcwt")
            cgb_t = pc.tile([128, 2], F32, tag="cgb")
            fcb_s = pc.tile([128, 2], F32, tag="fcbs")
            with tc.For_i(0, smin(any_on, 1), 1):
                nc.sync.dma_start(out=wnat_t[:, 0:9 * C], in_=wnat[0])
                nc.sync.dma_start(out=wnat_t[:, 9 * C:2 * 9 * C], in_=wnat[1])
                nc.sync.dma_start(out=cgw_t[:, 0:9 * C], in_=cgw[0])
                nc.sync.dma_start(out=cgw_t[:, 9 * C:2 * 9 * C], in_=cgw[1])
                nc.sync.dma_start(out=fcwt_t[:, 0:C], in_=fcwt[0])
                nc.sync.dma_start(out=fcwt_t[:, C:2 * C], in_=fcwt[1])
                nc.sync.dma_start(out=cgb_t[:], in_=cgb[:])
                nc.sync.dma_start(out=fcb_s[:], in_=fcb[:])
                # mask threshold works on un-normalized GAP sums: scale bias by
                # the 27x27 GAP count instead of dividing the sums
                nc.vector.tensor_scalar_mul(fcb_s[:], fcb_s[:], 729.0)

            out_rows = outp[:].rearrange("a c n -> (a c) n")

            # ---- per-sample gated heavy path ----
            for si in range(BS):
                with tc.For_i(0, l_vals[si], 1):
                    # -- channel-gate conv (stride-2 valid 3x3) + GAP --
                    g2 = pw.tile([128, 2], F32, tag="g2")
                    for cb in range(2):
                        accs = []
                        for (r0, r1) in G_CHUNKS:
                            rows = r1 - r0
                            pgc = pp.tile([128, rows * G_COLS], F32, tag="conv")
                            first = True
                            for tap in range(9):
                                dy, dx = tap // 3, tap % 3
                                for kb in range(2):
                                    xv = xtile(si, kb)[:, 0:PHW].rearrange(
                                        "p (h w) -> p h w", h=PH)
                                    rhs = xv[:, 2 * r0 + dy + 1:
                                             2 * (r1 - 1) + dy + 2:2,
                                             dx + 1:min(dx + 1 + 2 * G_COLS, PW):2]
                                    nc.tensor.matmul(
                                        pgc[:],
                                        cgw_t[:, kb * 9 * C + tap * C + cb * 128:
                                              kb * 9 * C + tap * C + cb * 128 + 128],
                                        rhs,
                                        start=first, stop=(tap == 8 and kb == 1))
                                    first = False
                            scr = pw.tile([128, 14 * G_COLS], F32, tag="gscr")
                            acc = pw.tile([128, 1], F32, tag=f"gacc{len(accs)}")
                            pv = pgc[:].rearrange("p (r c) -> p r c", c=G_COLS)
                            sv = scr[:].rearrange("p (r c) -> p r c", c=G_COLS)
                            nc.scalar.activation(sv[:, 0:rows, 0:27],
                                                 pv[:, :, 0:27], AF.Relu,
                                                 bias=cgb_t[:, cb:cb + 1],
                                                 accum_out=acc[:])
                            accs.append(acc)
                        nc.vector.tensor_tensor(out=g2[:, cb:cb + 1],
                                                in0=accs[0][:], in1=accs[1][:],
                                                op=ALU.add)

                    # -- fc -> binary mask per channel --
                    m_t = pw.tile([128, 2], F32, tag="mt")
                    for cbm in range(2):
                        pf = pp.tile([128, 1], F32, tag="small")
                        for kb in range(2):
                            nc.tensor.matmul(
                                pf[:],
                                fcwt_t[:, kb * C + cbm * 128:kb * C + cbm * 128 + 128],
                                g2[:, kb:kb + 1],
                                start=(kb == 0), stop=(kb == 1))
                        nc.vector.scalar_tensor_tensor(
                            out=m_t[:, cbm:cbm + 1], in0=pf[:],
                            scalar=fcb_s[:, cbm:cbm + 1], in1=zeros128[:],
                            op0=ALU.add, op1=ALU.is_gt)

                    # -- n = #active, exclusive prefix positions, one-hot S --
                    pn = pp.tile([1, 1], F32, tag="small")
                    for cb in range(2):
                        nc.tensor.matmul(pn[:], ones_t[:, 0:1], m_t[:, cb:cb + 1],
                                         start=(cb == 0), stop=(cb == 1))
                    n_sb = pw.tile([1, 1], F32, tag="nsb")
                    nc.scalar.activation(n_sb[:], pn[:], AF.Copy)
                    n_i32 = pw.tile([1, 2], I32, tag="ni32")
                    nc.vector.tensor_copy(out=n_i32[:, 0:1], in_=n_sb[:])
                    n2_sb = pw.tile([1, 1], F32, tag="n2sb")
                    nc.vector.tensor_scalar_sub(n2_sb[:], n_sb[:], 128.0)
                    nc.vector.tensor_copy(out=n_i32[:, 1:2], in_=n2_sb[:])
                    n_bc = pw.tile([128, 1], F32, tag="nbc")
                    nc.gpsimd.partition_broadcast(n_bc[:], n_sb[:])

                    pos_sb = pw.tile([128, 2], F32, tag="pos")
                    pp0 = pp.tile([128, 1], F32, tag="small")
                    nc.tensor.matmul(pp0[:], ucon_t[:], m_t[:, 0:1],
                                     start=True, stop=True)
                    nc.scalar.activation(pos_sb[:, 0:1], pp0[:], AF.Copy)
                    pp1 = pp.tile([128, 1], F32, tag="small")
                    nc.tensor.matmul(pp1[:], ones_t[:], m_t[:, 0:1],
                                     start=True, stop=False)
                    nc.tensor.matmul(pp1[:], ucon_t[:], m_t[:, 1:2],
                                     start=False, stop=True)
                    nc.scalar.activation(pos_sb[:, 1:2], pp1[:], AF.Copy)

                    s_ts = []
                    for cb in range(2):
                        s_t = pw.tile([128, 256], F32, tag=f"s{cb}")
                        nc.vector.tensor_scalar(
                            out=s_t[:].bitcast(F32R), in0=j_t[:],
                            scalar1=pos_sb[:, cb:cb + 1],
                            scalar2=None, op0=ALU.is_equal)
                        nc.vector.tensor_scalar(
                            out=s_t[:].bitcast(F32R), in0=s_t[:],
                            scalar1=m_t[:, cb:cb + 1], scalar2=None, op0=ALU.mult)
                        s_ts.append(s_t)

                    nc.vector.tensor_copy(out=dbg_t[:, 4 + si:5 + si], in_=g2[:, 0:1])
                    nc.vector.tensor_copy(out=dbg_t[:, 8 + si:9 + si], in_=m_t[:, 0:1])
                    nc.vector.tensor_copy(out=dbg_t[:, 12 + si:13 + si], in_=pos_sb[:, 0:1])
                    # -- scatter row indices (pads pushed out of bounds) --
                    idx_i32 = pw.tile([128, 2], I32, tag="idxi")
                    for j in range(2):
                        pi = pp.tile([128, 1], F32, tag="small")
                        for cb in range(2):
                            nc.tensor.matmul(pi[:],
                                             s_ts[cb][:, j * 128:(j + 1) * 128],
                                             cvec_t[:, cb:cb + 1],
                                             start=(cb == 0), stop=(cb == 1))
                        cmp = pw.tile([128, 1], F32, tag="cmp")
                        if j == 0:
                            nc.vector.tensor_scalar(
                                out=cmp[:], in0=cvec_t[:, 0:1],
                                scalar1=n_bc[:, 0:1], scalar2=None, op0=ALU.is_ge)
                        else:
                            nc.vector.tensor_scalar(
                                out=cmp[:], in0=cvec_t[:, 1:2],
                                scalar1=n_bc[:, 0:1], scalar2=None, op0=ALU.is_ge)
                        idxf = pw.tile([128, 1], F32, tag="idxf")
                        nc.vector.scalar_tensor_tensor(
                            out=idxf[:], in0=cmp[:], scalar=4096.0, in1=pi[:],
                            op0=ALU.mult, op1=ALU.add)
                        nc.vector.tensor_scalar(
                            out=idxf[:], in0=idxf[:], scalar1=float(si * C),
                            scalar2=None, op0=ALU.add)
                        nc.vector.tensor_copy(out=idx_i32[:, j:j + 1], in_=idxf[:])

                    # -- PE-side weight gather: selw[:, wi*256+p] = W[idx_p, cin] --
                    selw = pw.tile([128, 18 * 256], F32R, tag="selw")
                    for wi in range(18):
                        tap, kb = wi // 2, wi % 2
                        ps = pp.tile([128, 256], F32, tag="sel")
                        for cb in range(2):
                            nc.tensor.matmul(
                                ps[:],
                                wnat_t[:, cb * 9 * C + tap * C + kb * 128:
                                       cb * 9 * C + tap * C + kb * 128 + 128],
                                s_ts[cb][:].bitcast(F32R),
                                start=(cb == 0), stop=(cb == 1))
                        nc.vector.tensor_copy(
                            out=selw[:, wi * 256:(wi + 1) * 256], in_=ps[:])

                    # -- compact conv + extract + scatter, per 128-row block --
                    n_val = nc.values_load(n_i32[0:1, 0:1], engines=ENGINES,
                                           min_val=0, max_val=256,
                                           skip_runtime_bounds_check=True)
                    n2_val = nc.values_load(n_i32[0:1, 1:2], engines=ENGINES,
                                            min_val=-256, max_val=128,
                                            skip_runtime_bounds_check=True)
                    for j, trip in ((0, smin(n_val, 1)), (1, smin(n2_val, 1))):
                        with tc.For_i(0, trip, 1):
                            banks = [pp.tile([128, CH_N], F32, tag="conv",
                                              name=f"bank{_k}")
                                     for _k in range(NCHUNK)]
                            for wi in range(18):
                                tap, kb = wi // 2, wi % 2
                                dy, dx = tap // 3, tap % 3
                                xt = xtile(si, kb)
                                for k in range(NCHUNK):
                                    off = (CH_ROWS * k + dy) * PH + dx
                                    nc.tensor.matmul(
                                        banks[k][:],
                                        selw[:, wi * 256 + j * 128:
                                             wi * 256 + j * 128 + 128],
                                        xt[:, off:off + CH_N],
                                        start=(wi == 0), stop=(wi == 17))
                            stg = pw.tile([128, HW], F32, tag="stg")
                            for k in range(NCHUNK):
                                bv = banks[k][:].rearrange(
                                    "p (r c) -> p r c", c=PH)
                                sv = stg[:].rearrange("p (r c) -> p r c", c=W)
                                nc.scalar.activation(
                                    sv[:, k * CH_ROWS:(k + 1) * CH_ROWS, :],
                                    bv[:, :, 0:W], AF.Copy)
                            nc.gpsimd.indirect_dma_start(
                                out=out_rows,
                                out_offset=bass.IndirectOffsetOnAxis(
                                    ap=idx_i32[:, j:j + 1], axis=0),
                                in_=stg[:], in_offset=None,
                                bounds_check=BS * C - 1, oob_is_err=False)
            nc.sync.dma_start(out=dbg[:], in_=dbg_t[:])

    nc.compile()
    return nc


def _host_layouts(inputs):
    conv_w = np.asarray(inputs["conv_w"], np.float32)
    cg_conv_w = np.asarray(inputs["cg_conv_w"], np.float32)
    cg_fc_w = np.asarray(inputs["cg_fc_w"], np.float32)
    lg_conv_w = np.asarray(inputs["lg_conv_w"], np.float32)
    w_ih = np.asarray(inputs["lstm_w_ih"], np.float32)

    # wnat[cb][cout, tap*256+cin] = conv_w[cb*128+cout, cin, dy, dx]
    wn = conv_w.transpose(0, 2, 3, 1).reshape(C, 9 * C)
    wnat = np.ascontiguousarray(wn.reshape(2, 128, 9 * C))
    # cgw[kb][cin, tap*256+cout] = cg_conv_w[cout, kb*128+cin, dy, dx]
    cg = cg_conv_w.transpose(1, 2, 3, 0).reshape(C, 9 * C)
    cgw = np.ascontiguousarray(cg.reshape(2, 128, 9 * C))
    # fcwt[kb][k, c] = cg_fc_w[c, kb*128+k]
    fcwt = np.ascontiguousarray(cg_fc_w.T.reshape(2, 128, C))
    # lgwt[kb][k, m] = lg_conv_w[m, kb*128+k]
    lgwt = np.ascontiguousarray(
        lg_conv_w.reshape(LSTM_H, C).T.reshape(2, 128, LSTM_H))
    wiht = np.concatenate(
        [w_ih.T, (np.asarray(inputs["lstm_b_ih"], np.float32)
                  + np.asarray(inputs["lstm_b_hh"], np.float32))[None, :]],
        axis=0)
    wiht = np.ascontiguousarray(wiht)

    cgb = np.ascontiguousarray(
        np.asarray(inputs["cg_conv_b"], np.float32).reshape(2, 128).T)
    fcb = np.ascontiguousarray(
        np.asarray(inputs["cg_fc_b"], np.float32).reshape(2, 128).T)

    u = np.triu(np.ones((128, 128), np.float32), k=1)
    jc = np.tile(np.arange(256, dtype=np.float32)[None, :], (128, 1))
    cv = np.stack([np.arange(128, dtype=np.float32),
                   np.arange(128, 256, dtype=np.float32)], axis=1)

    return {
        "wnat": wnat, "cgw": cgw, "fcwt": fcwt, "lgwt": lgwt, "wiht": wiht,
        "lgfc": np.ascontiguousarray(
            np.asarray(inputs["lg_fc_w"], np.float32).reshape(1, LSTM_H)),
        "cgb": cgb, "fcb": fcb,
        "lgb": np.ascontiguousarray(
            np.asarray(inputs["lg_conv_b"], np.float32).reshape(LSTM_H, 1)),
        "lfb": np.ascontiguousarray(
            np.asarray(inputs["lg_fc_b"], np.float32).reshape(1, 1)),
        "ucon": np.ascontiguousarray(u),
        "onesk": np.ones((128, 128), np.float32),
        "jcon": np.ascontiguousarray(jc),
        "cvec": np.ascontiguousarray(cv),
    }


def kernel(**inputs):
    if "nc" not in _CACHE:
        _CACHE["nc"] = _build()
    nc = _CACHE["nc"]

    x = np.asarray(inputs["x"], np.float32)
    shared = _host_layouts(inputs)
    in_maps = []
    for core in range(NCORES):
        m = dict(shared)
        m["x"] = np.ascontiguousarray(x[core * BS:(core + 1) * BS])
        in_maps.append(m)

    trace = bool(int(os.environ.get("BASS_KERNEL_TRACE", "0")))
    kw = {}
    if trace:
        from trn_agent_boot.trn_boot import _ntff_profile_via_ctypes
        import antenv.axon_hooks as ah
        ah.set_axon_ntff_profile_hook(
            _ntff_profile_via_ctypes("/opt/axon/libaxon_pjrt.so"))
        import tempfile
        base = os.environ.get("BASS_KERNEL_TRACE_DIR", "/tmp/adaptconv_trace")
        os.makedirs(base, exist_ok=True)
        kw = dict(trace=True, tmpdir=tempfile.mkdtemp(dir=base))

    res = run_bass_kernel_spmd(nc, in_maps, core_ids=list(range(NCORES)), **kw)
    _CACHE["last_exec_time_ns"] = res.exec_time_ns

    _CACHE["dbg"] = [res.results[i].get("dbg") for i in range(NCORES)]
    out = np.concatenate(
        [res.results[i]["out"].reshape(BS, C, H, W) for i in range(NCORES)],
        axis=0)
    return out


# revision 14
# speedup vs baseline: 1.1786x; 1.1786x over previous
"""AdaptConv2d Trainium2 kernel: 8-core data-parallel, gate-driven sparse conv.

Computes, per sample b:
  layer_bit = (LSTM-gate pre-activation > 0)
  if layer_bit:  channel mask m_c = (channel-gate fc pre-activation > 0)
                 out[c] = conv3x3(x)[c] if m_c else x[c]
  else:          out = x

Device strategy per core (4 samples):
  - x loaded into zero-padded (58x58) SBUF images, f32r (tf32) typed for the
    TensorEngine fast path; bits are untouched so pass-through output is exact.
  - Layer gate (GAP + 1x1-conv + single-step LSTM + fc) in true fp32 for all 4
    samples, branch-free.
  - Per sample, a 0/1-trip For_i (trip = layer bit) guards the heavy work:
    stride-2 channel-gate conv (tf32 matmuls, fp32 accum), fp32 fc -> binary
    mask, prefix-sum -> one-hot selection matrix S, PE-side weight gather
    (W^T @ S), compact conv over only ceil(n_active/128) 128-channel blocks
    (inner 0/1-trip For_i for the second block), and an indirect row-scatter
    of conv rows into the output (out-of-bounds pad rows silently dropped).
  - Unconditional default write out = x covers inactive channels/samples.
"""

import os
import sys
import types

sys.path.insert(0, "/opt/trn_rl_repo")

import numpy as np

# antenv.axon_hooks is missing from this image; inject a minimal stand-in so
# run_bass_kernel_spmd's trace path imports cleanly (used only when tracing).
try:
    import antenv  # noqa: F401

    if "antenv.axon_hooks" not in sys.modules:
        _m = types.ModuleType("antenv.axon_hooks")
        _h = [None]
        _m.set_axon_ntff_profile_hook = lambda hook: _h.__setitem__(0, hook)
        _m.get_axon_ntff_profile_hook = lambda: _h[0]
        sys.modules["antenv.axon_hooks"] = _m
        antenv.axon_hooks = _m
except Exception:
    pass

import concourse.bass as bass
import concourse.mybir as mybir
from concourse import bacc
from concourse.expressions import smin
from concourse.tile import TileContext
from concourse.bass_utils import run_bass_kernel_spmd

F32 = mybir.dt.float32
F32R = mybir.dt.float32r
I32 = mybir.dt.int32
AF = mybir.ActivationFunctionType
ALU = mybir.AluOpType

B, C, H, W = 32, 256, 56, 56
NCORES = 8
BS = B // NCORES          # samples per core
HW = H * W                # 3136
PH, PW = H + 2, W + 2     # 58x58 padded image
PHW = PH * PW             # 3364
XT_COLS = PHW + 4         # tail pad: edge-tap reads run 2 past the image
LSTM_H = 10
ENGINES = list(mybir.ALL_ENGINES)

# main-conv spatial chunking: 7 chunks x 8 valid rows; each chunk is a
# contiguous 464-wide span of the padded image (includes L/R pad cols, whose
# outputs are junk and excluded at extraction time)
NCHUNK = 7
CH_ROWS = 8
CH_N = CH_ROWS * PH       # 464

# channel-gate conv: 27x27 valid outputs, row-chunks of 14/13, 28 cols (28th
# col junk so the fp32r moving operand has an even innermost count)
G_CHUNKS = ((0, 14), (14, 27))
G_COLS = 28

_CACHE = {}


def _build():
    nc = bacc.Bacc(None, target_bir_lowering=False)

    xp = nc.declare_dram_parameter("x", [BS, C, H, W], F32, isOutput=False)
    outp = nc.declare_dram_parameter("out", [BS, C, HW], F32, isOutput=True)
    wnat = nc.declare_dram_parameter("wnat", [2, 128, 9 * C], F32R, isOutput=False)
    cgw = nc.declare_dram_parameter("cgw", [2, 128, 9 * C], F32R, isOutput=False)
    fcwt = nc.declare_dram_parameter("fcwt", [2, 128, C], F32, isOutput=False)
    lgwt = nc.declare_dram_parameter("lgwt", [2, 128, LSTM_H], F32, isOutput=False)
    wiht = nc.declare_dram_parameter("wiht", [LSTM_H + 1, 4 * LSTM_H], F32, isOutput=False)
    lgfc = nc.declare_dram_parameter("lgfc", [1, LSTM_H], F32, isOutput=False)
    cgb = nc.declare_dram_parameter("cgb", [128, 2], F32, isOutput=False)
    fcb = nc.declare_dram_parameter("fcb", [128, 2], F32, isOutput=False)
    lgb = nc.declare_dram_parameter("lgb", [LSTM_H, 1], F32, isOutput=False)
    lfb = nc.declare_dram_parameter("lfb", [1, 1], F32, isOutput=False)
    ucon = nc.declare_dram_parameter("ucon", [128, 128], F32, isOutput=False)
    onesk = nc.declare_dram_parameter("onesk", [128, 128], F32, isOutput=False)
    jcon = nc.declare_dram_parameter("jcon", [128, 2 * 128], F32, isOutput=False)
    cvec = nc.declare_dram_parameter("cvec", [128, 2], F32, isOutput=False)
    dbg = nc.declare_dram_parameter("dbg", [128, 16], F32, isOutput=True)

    with TileContext(nc) as tc:
        with tc.tile_pool(name="sbuf", bufs=1) as pc, \
             tc.tile_pool(name="work", bufs=1) as pw, \
             tc.tile_pool(name="psum", bufs=1, space="PSUM") as pp:

            # ---- constants / weights resident in SBUF ----
            ucon_t = pc.tile([128, 128], F32, tag="ucon")
            nc.sync.dma_start(out=ucon_t[:], in_=ucon[:])
            ones_t = pc.tile([128, 128], F32, tag="ones")
            nc.sync.dma_start(out=ones_t[:], in_=onesk[:])
            j_t = pc.tile([128, 256], F32, tag="jcon")
            nc.sync.dma_start(out=j_t[:], in_=jcon[:])
            cvec_t = pc.tile([128, 2], F32, tag="cvec")
            nc.sync.dma_start(out=cvec_t[:], in_=cvec[:])
            lgwt_t = pc.tile([128, 2 * LSTM_H], F32, tag="lgwt")
            nc.sync.dma_start(out=lgwt_t[:, 0:LSTM_H], in_=lgwt[0])
            nc.sync.dma_start(out=lgwt_t[:, LSTM_H:2 * LSTM_H], in_=lgwt[1])
            wiht_t = pc.tile([LSTM_H + 1, 4 * LSTM_H], F32, tag="wiht")
            nc.sync.dma_start(out=wiht_t[:], in_=wiht[:])
            lgb_t = pc.tile([LSTM_H, 1], F32, tag="lgb")
            nc.sync.dma_start(out=lgb_t[:], in_=lgb[:])
            lgfc_t = pc.tile([1, LSTM_H], F32, tag="lgfc")
            nc.sync.dma_start(out=lgfc_t[:], in_=lgfc[:])
            lfb_t = pc.tile([1, 1], F32, tag="lfb")
            nc.sync.dma_start(out=lfb_t[:], in_=lfb[:])

            # broadcast tiny layer-gate constants across the 4 sample rows
            lrow_b = pc.tile([BS, LSTM_H], F32, tag="lrowb")
            nc.gpsimd.partition_broadcast(lrow_b[:], lgfc_t[:])
            fb_b = pc.tile([BS, 1], F32, tag="fbb")
            nc.gpsimd.partition_broadcast(fb_b[:], lfb_t[:])
            zeros4 = pc.tile([BS, 1], F32, tag="z4")
            nc.vector.memset(zeros4[:], 0.0)
            zeros128 = pc.tile([128, 1], F32, tag="z128")
            nc.vector.memset(zeros128[:], 0.0)

            # ---- x streamed contiguously: exact pass-through + exact GAP ----
            g1 = pc.tile([128, 2 * BS], F32, tag="g1")  # GAP sums, col = kb*BS+si
            for si in range(BS):
                for kb in range(2):
                    xu = pw.tile([128, HW], F32, tag="xu", bufs=3)
                    nc.sync.dma_start(out=xu[:],
                                      in_=xp[si, kb * 128:(kb + 1) * 128]
                                      .rearrange("p a b -> p (a b)"))
                    nc.vector.tensor_reduce(
                        out=g1[:, kb * BS + si:kb * BS + si + 1],
                        in_=xu[:], axis=mybir.AxisListType.X, op=ALU.add)
                    nc.sync.dma_start(out=outp[si, kb * 128:(kb + 1) * 128],
                                      in_=xu[:])

            ph = pp.tile([LSTM_H, BS], F32, tag="small")
            for kb in range(2):
                nc.tensor.matmul(ph[:], lgwt_t[:, kb * LSTM_H:(kb + 1) * LSTM_H],
                                 g1[:, kb * BS:(kb + 1) * BS],
                                 start=(kb == 0), stop=(kb == 1))
            htile = pc.tile([LSTM_H + 1, BS], F32, tag="htile")
            nc.sync.dma_start(out=htile[LSTM_H:LSTM_H + 1, :],
                              in_=onesk[0:1, 0:BS])
            nc.scalar.activation(htile[0:LSTM_H, :], ph[:], AF.Relu,
                                 bias=lgb_t[:, 0:1], scale=1.0 / HW)
            pg = pp.tile([BS, 4 * LSTM_H], F32, tag="small")
            nc.tensor.matmul(pg[:], htile[:], wiht_t[:], start=True, stop=True)
            lwork = pc.tile([BS, 4 * LSTM_H], F32, tag="lwork")
            nc.scalar.activation(lwork[:, 0:LSTM_H], pg[:, 0:LSTM_H], AF.Sigmoid)
            nc.scalar.activation(lwork[:, 3 * LSTM_H:4 * LSTM_H],
                                 pg[:, 3 * LSTM_H:4 * LSTM_H], AF.Sigmoid)
            nc.scalar.activation(lwork[:, 2 * LSTM_H:3 * LSTM_H],
                                 pg[:, 2 * LSTM_H:3 * LSTM_H], AF.Tanh)
            cbuf = pc.tile([BS, LSTM_H], F32, tag="cbuf")
            nc.vector.tensor_tensor(out=cbuf[:], in0=lwork[:, 0:LSTM_H],
                                    in1=lwork[:, 2 * LSTM_H:3 * LSTM_H],
                                    op=ALU.mult)
            ebuf = pc.tile([BS, LSTM_H], F32, tag="ebuf")
            nc.scalar.activation(ebuf[:], cbuf[:], AF.Tanh)
            hsb = pc.tile([BS, LSTM_H], F32, tag="hsb")
            nc.vector.tensor_tensor(out=hsb[:], in0=lwork[:, 3 * LSTM_H:4 * LSTM_H],
                                    in1=ebuf[:], op=ALU.mult)
            prod = pc.tile([BS, LSTM_H], F32, tag="prod")
            nc.vector.tensor_tensor(out=prod[:], in0=hsb[:], in1=lrow_b[:],
                                    op=ALU.mult)
            lpre = pc.tile([BS, 1], F32, tag="lpre")
            nc.vector.tensor_reduce(out=lpre[:], in_=prod[:],
                                    axis=mybir.AxisListType.X, op=ALU.add)
            l_bin = pc.tile([BS, 1], F32, tag="lbin")
            nc.vector.scalar_tensor_tensor(out=l_bin[:], in0=lpre[:],
                                           scalar=fb_b[:, 0:1], in1=zeros4[:],
                                           op0=ALU.add, op1=ALU.is_gt)
            dbg_t = pc.tile([128, 16], F32, tag="dbg")
            nc.vector.memset(dbg_t[:], 0.0)
            nc.vector.tensor_copy(out=dbg_t[0:BS, 0:1], in_=lpre[:])
            nc.vector.tensor_copy(out=dbg_t[0:BS, 1:2], in_=l_bin[:])
            nc.vector.tensor_copy(out=dbg_t[0:128, 2:4], in_=g1[:, 0:2 * BS:4])
            l_i32 = pc.tile([BS, 1], I32, tag="li32")
            nc.vector.tensor_copy(out=l_i32[:], in_=l_bin[:])

            l_vals = [nc.values_load(l_i32[si:si + 1, 0:1], engines=ENGINES,
                                     min_val=0, max_val=1,
                                     skip_runtime_bounds_check=True)
                      for si in range(BS)]

            # gate/conv weights only needed if any sample is layer-on
            any_on = l_vals[0] + l_vals[1] + l_vals[2] + l_vals[3]
            wnat_t = pc.tile([128, 2 * 9 * C], F32R, tag="wnat")
            cgw_t = pc.tile([128, 2 * 9 * C], F32R, tag="cgw")
            fcwt_t = pc.tile([128, 2 * C], F32, tag="fcwt")
            cgb_t = pc.tile([128, 2], F32, tag="cgb")
            fcb_s = pc.tile([128, 2], F32, tag="fcbs")
            with tc.For_i(0, smin(any_on, 1), 1):
                nc.sync.dma_start(out=wnat_t[:, 0:9 * C], in_=wnat[0])
                nc.sync.dma_start(out=wnat_t[:, 9 * C:2 * 9 * C], in_=wnat[1])
                nc.sync.dma_start(out=cgw_t[:, 0:9 * C], in_=cgw[0])
                nc.sync.dma_start(out=cgw_t[:, 9 * C:2 * 9 * C], in_=cgw[1])
                nc.sync.dma_start(out=fcwt_t[:, 0:C], in_=fcwt[0])
                nc.sync.dma_start(out=fcwt_t[:, C:2 * C], in_=fcwt[1])
                nc.sync.dma_start(out=cgb_t[:], in_=cgb[:])
                nc.sync.dma_start(out=fcb_s[:], in_=fcb[:])
                # mask threshold works on un-normalized GAP sums: scale bias by
                # the 27x27 GAP count instead of dividing the sums
                nc.vector.tensor_scalar_mul(fcb_s[:], fcb_s[:], 729.0)

            out_rows = outp[:].rearrange("a c n -> (a c) n")

            # ---- per-sample gated heavy path ----
            for si in range(BS):
                with tc.For_i(0, l_vals[si], 1):
                    # re-load this sample's x and build zero-padded f32r images
                    xts = []
                    for kb in range(2):
                        xr = pw.tile([128, HW], F32, tag="xr", bufs=2)
                        nc.sync.dma_start(out=xr[:],
                                          in_=xp[si, kb * 128:(kb + 1) * 128]
                                          .rearrange("p a b -> p (a b)"))
                        xt = pw.tile([128, XT_COLS], F32R, tag=f"xpad{kb}")
                        xv = xt[:, 0:PHW].rearrange("p (h w) -> p h w", h=PH)
                        nc.vector.memset(xv[:, 0:1, :].bitcast(F32), 0.0)
                        nc.vector.memset(xv[:, PH - 1:PH, :].bitcast(F32), 0.0)
                        nc.vector.memset(xv[:, :, 0:1].bitcast(F32), 0.0)
                        nc.vector.memset(xv[:, :, PW - 1:PW].bitcast(F32), 0.0)
                        nc.vector.memset(xt[:, PHW:XT_COLS].bitcast(F32), 0.0)
                        eng = nc.vector if kb == 0 else nc.scalar
                        if kb == 0:
                            nc.vector.tensor_copy(
                                out=xv[:, 1:PH - 1, 1:PW - 1],
                                in_=xr[:].rearrange("p (a b) -> p a b", a=H))
                        else:
                            nc.scalar.activation(
                                xv[:, 1:PH - 1, 1:PW - 1],
                                xr[:].rearrange("p (a b) -> p a b", a=H),
                                AF.Copy)
                        xts.append(xt)

                    def xtile(si_, kb_):
                        return xts[kb_]

                    # -- channel-gate conv (stride-2 valid 3x3) + GAP --
                    g2 = pw.tile([128, 2], F32, tag="g2")
                    for cb in range(2):
                        accs = []
                        for (r0, r1) in G_CHUNKS:
                            rows = r1 - r0
                            pgc = pp.tile([128, rows * G_COLS], F32, tag="conv")
                            first = True
                            for tap in range(9):
                                dy, dx = tap // 3, tap % 3
                                for kb in range(2):
                                    xv = xtile(si, kb)[:, 0:PHW].rearrange(
                                        "p (h w) -> p h w", h=PH)
                                    rhs = xv[:, 2 * r0 + dy + 1:
                                             2 * (r1 - 1) + dy + 2:2,
                                             dx + 1:min(dx + 1 + 2 * G_COLS, PW):2]
                                    nc.tensor.matmul(
                                        pgc[:],
                                        cgw_t[:, kb * 9 * C + tap * C + cb * 128:
                                              kb * 9 * C + tap * C + cb * 128 + 128],
                                        rhs,
                                        start=first, stop=(tap == 8 and kb == 1))
                                    first = False
                            scr = pw.tile([128, 14 * G_COLS], F32, tag="gscr")
                            acc = pw.tile([128, 1], F32, tag=f"gacc{len(accs)}")
                            pv = pgc[:].rearrange("p (r c) -> p r c", c=G_COLS)
                            sv = scr[:].rearrange("p (r c) -> p r c", c=G_COLS)
                            nc.scalar.activation(sv[:, 0:rows, 0:27],
                                                 pv[:, :, 0:27], AF.Relu,
                                                 bias=cgb_t[:, cb:cb + 1],
                                                 accum_out=acc[:])
                            accs.append(acc)
                        nc.vector.tensor_tensor(out=g2[:, cb:cb + 1],
                                                in0=accs[0][:], in1=accs[1][:],
                                                op=ALU.add)

                    # -- fc -> binary mask per channel --
                    m_t = pw.tile([128, 2], F32, tag="mt")
                    for cbm in range(2):
                        pf = pp.tile([128, 1], F32, tag="small")
                        for kb in range(2):
                            nc.tensor.matmul(
                                pf[:],
                                fcwt_t[:, kb * C + cbm * 128:kb * C + cbm * 128 + 128],
                                g2[:, kb:kb + 1],
                                start=(kb == 0), stop=(kb == 1))
                        nc.vector.scalar_tensor_tensor(
                            out=m_t[:, cbm:cbm + 1], in0=pf[:],
                            scalar=fcb_s[:, cbm:cbm + 1], in1=zeros128[:],
                            op0=ALU.add, op1=ALU.is_gt)

                    # -- n = #active, exclusive prefix positions, one-hot S --
                    pn = pp.tile([1, 1], F32, tag="small")
                    for cb in range(2):
                        nc.tensor.matmul(pn[:], ones_t[:, 0:1], m_t[:, cb:cb + 1],
                                         start=(cb == 0), stop=(cb == 1))
                    n_sb = pw.tile([1, 1], F32, tag="nsb")
                    nc.scalar.activation(n_sb[:], pn[:], AF.Copy)
                    n_i32 = pw.tile([1, 2], I32, tag="ni32")
                    nc.vector.tensor_copy(out=n_i32[:, 0:1], in_=n_sb[:])
                    n2_sb = pw.tile([1, 1], F32, tag="n2sb")
                    nc.vector.tensor_scalar_sub(n2_sb[:], n_sb[:], 128.0)
                    nc.vector.tensor_copy(out=n_i32[:, 1:2], in_=n2_sb[:])
                    n_bc = pw.tile([128, 1], F32, tag="nbc")
                    nc.gpsimd.partition_broadcast(n_bc[:], n_sb[:])

                    pos_sb = pw.tile([128, 2], F32, tag="pos")
                    pp0 = pp.tile([128, 1], F32, tag="small")
                    nc.tensor.matmul(pp0[:], ucon_t[:], m_t[:, 0:1],
                                     start=True, stop=True)
                    nc.scalar.activation(pos_sb[:, 0:1], pp0[:], AF.Copy)
                    pp1 = pp.tile([128, 1], F32, tag="small")
                    nc.tensor.matmul(pp1[:], ones_t[:], m_t[:, 0:1],
                                     start=True, stop=False)
                    nc.tensor.matmul(pp1[:], ucon_t[:], m_t[:, 1:2],
                                     start=False, stop=True)
                    nc.scalar.activation(pos_sb[:, 1:2], pp1[:], AF.Copy)

                    s_ts = []
                    for cb in range(2):
                        s_t = pw.tile([128, 256], F32, tag=f"s{cb}")
                        nc.vector.tensor_scalar(
                            out=s_t[:].bitcast(F32R), in0=j_t[:],
                            scalar1=pos_sb[:, cb:cb + 1],
                            scalar2=None, op0=ALU.is_equal)
                        nc.vector.tensor_scalar(
                            out=s_t[:].bitcast(F32R), in0=s_t[:],
                            scalar1=m_t[:, cb:cb + 1], scalar2=None, op0=ALU.mult)
                        s_ts.append(s_t)

                    nc.vector.tensor_copy(out=dbg_t[:, 4 + si:5 + si], in_=g2[:, 0:1])
                    nc.vector.tensor_copy(out=dbg_t[:, 8 + si:9 + si], in_=m_t[:, 0:1])
                    nc.vector.tensor_copy(out=dbg_t[:, 12 + si:13 + si], in_=pos_sb[:, 0:1])
                    # -- scatter row indices (pads pushed out of bounds) --
                    idx_i32 = pw.tile([128, 2], I32, tag="idxi")
                    for j in range(2):
                        pi = pp.tile([128, 1], F32, tag="small")
                        for cb in range(2):
                            nc.tensor.matmul(pi[:],
                                             s_ts[cb][:, j * 128:(j + 1) * 128],
                                             cvec_t[:, cb:cb + 1],
                                             start=(cb == 0), stop=(cb == 1))
                        cmp = pw.tile([128, 1], F32, tag="cmp")
                        if j == 0:
                            nc.vector.tensor_scalar(
                                out=cmp[:], in0=cvec_t[:, 0:1],
                                scalar1=n_bc[:, 0:1], scalar2=None, op0=ALU.is_ge)
                        else:
                            nc.vector.tensor_scalar(
                                out=cmp[:], in0=cvec_t[:, 1:2],
                                scalar1=n_bc[:, 0:1], scalar2=None, op0=ALU.is_ge)
                        idxf = pw.tile([128, 1], F32, tag="idxf")
                        nc.vector.scalar_tensor_tensor(
                            out=idxf[:], in0=cmp[:], scalar=4096.0, in1=pi[:],
                            op0=ALU.mult, op1=ALU.add)
                        nc.vector.tensor_scalar(
                            out=idxf[:], in0=idxf[:], scalar1=float(si * C),
                            scalar2=None, op0=ALU.add)
                        nc.vector.tensor_copy(out=idx_i32[:, j:j + 1], in_=idxf[:])

                    # -- PE-side weight gather: selw[:, wi*256+p] = W[idx_p, cin] --
                    selw = pw.tile([128, 18 * 256], F32R, tag="selw")
                    for wi in range(18):
                        tap, kb = wi // 2, wi % 2
                        ps = pp.tile([128, 256], F32, tag="sel")
                        for cb in range(2):
                            nc.tensor.matmul(
                                ps[:],
                                wnat_t[:, cb * 9 * C + tap * C + kb * 128:
                                       cb * 9 * C + tap * C + kb * 128 + 128],
                                s_ts[cb][:].bitcast(F32R),
                                start=(cb == 0), stop=(cb == 1))
                        nc.vector.tensor_copy(
                            out=selw[:, wi * 256:(wi + 1) * 256], in_=ps[:])

                    # -- compact conv + extract + scatter, per 128-row block --
                    n_val = nc.values_load(n_i32[0:1, 0:1], engines=ENGINES,
                                           min_val=0, max_val=256,
                                           skip_runtime_bounds_check=True)
                    n2_val = nc.values_load(n_i32[0:1, 1:2], engines=ENGINES,
                                            min_val=-256, max_val=128,
                                            skip_runtime_bounds_check=True)
                    for j, trip in ((0, smin(n_val, 1)), (1, smin(n2_val, 1))):
                        with tc.For_i(0, trip, 1):
                            banks = [pp.tile([128, CH_N], F32, tag="conv",
                                              name=f"bank{_k}")
                                     for _k in range(NCHUNK)]
                            for wi in range(18):
                                tap, kb = wi // 2, wi % 2
                                dy, dx = tap // 3, tap % 3
                                xt = xtile(si, kb)
                                for k in range(NCHUNK):
                                    off = (CH_ROWS * k + dy) * PH + dx
                                    nc.tensor.matmul(
                                        banks[k][:],
                                        selw[:, wi * 256 + j * 128:
                                             wi * 256 + j * 128 + 128],
                                        xt[:, off:off + CH_N],
                                        start=(wi == 0), stop=(wi == 17))
                            stg = pw.tile([128, HW], F32, tag="stg")
                            for k in range(NCHUNK):
                                bv = banks[k][:].rearrange(
                                    "p (r c) -> p r c", c=PH)
                                sv = stg[:].rearrange("p (r c) -> p r c", c=W)
                                nc.scalar.activation(
                                    sv[:, k * CH_ROWS:(k + 1) * CH_ROWS, :],
                                    bv[:, :, 0:W], AF.Copy)
                            nc.gpsimd.indirect_dma_start(
                                out=out_rows,
                                out_offset=bass.IndirectOffsetOnAxis(
                                    ap=idx_i32[:, j:j + 1], axis=0),
                                in_=stg[:], in_offset=None,
                                bounds_check=BS * C - 1, oob_is_err=False)
            nc.sync.dma_start(out=dbg[:], in_=dbg_t[:])

    nc.compile()
    return nc


def _host_layouts(inputs):
    conv_w = np.asarray(inputs["conv_w"], np.float32)
    cg_conv_w = np.asarray(inputs["cg_conv_w"], np.float32)
    cg_fc_w = np.asarray(inputs["cg_fc_w"], np.float32)
    lg_conv_w = np.asarray(inputs["lg_conv_w"], np.float32)
    w_ih = np.asarray(inputs["lstm_w_ih"], np.float32)

    # wnat[cb][cout, tap*256+cin] = conv_w[cb*128+cout, cin, dy, dx]
    wn = conv_w.transpose(0, 2, 3, 1).reshape(C, 9 * C)
    wnat = np.ascontiguousarray(wn.reshape(2, 128, 9 * C))
    # cgw[kb][cin, tap*256+cout] = cg_conv_w[cout, kb*128+cin, dy, dx]
    cg = cg_conv_w.transpose(1, 2, 3, 0).reshape(C, 9 * C)
    cgw = np.ascontiguousarray(cg.reshape(2, 128, 9 * C))
    # fcwt[kb][k, c] = cg_fc_w[c, kb*128+k]
    fcwt = np.ascontiguousarray(cg_fc_w.T.reshape(2, 128, C))
    # lgwt[kb][k, m] = lg_conv_w[m, kb*128+k]
    lgwt = np.ascontiguousarray(
        lg_conv_w.reshape(LSTM_H, C).T.reshape(2, 128, LSTM_H))
    wiht = np.concatenate(
        [w_ih.T, (np.asarray(inputs["lstm_b_ih"], np.float32)
                  + np.asarray(inputs["lstm_b_hh"], np.float32))[None, :]],
        axis=0)
    wiht = np.ascontiguousarray(wiht)

    cgb = np.ascontiguousarray(
        np.asarray(inputs["cg_conv_b"], np.float32).reshape(2, 128).T)
    fcb = np.ascontiguousarray(
        np.asarray(inputs["cg_fc_b"], np.float32).reshape(2, 128).T)

    u = np.triu(np.ones((128, 128), np.float32), k=1)
    jc = np.tile(np.arange(256, dtype=np.float32)[None, :], (128, 1))
    cv = np.stack([np.arange(128, dtype=np.float32),
                   np.arange(128, 256, dtype=np.float32)], axis=1)

    return {
        "wnat": wnat, "cgw": cgw, "fcwt": fcwt, "lgwt": lgwt, "wiht": wiht,
        "lgfc": np.ascontiguousarray(
            np.asarray(inputs["lg_fc_w"], np.float32).reshape(1, LSTM_H)),
        "cgb": cgb, "fcb": fcb,
        "lgb": np.ascontiguousarray(
            np.asarray(inputs["lg_conv_b"], np.float32).reshape(LSTM_H, 1)),
        "lfb": np.ascontiguousarray(
            np.asarray(inputs["lg_fc_b"], np.float32).reshape(1, 1)),
        "ucon": np.ascontiguousarray(u),
        "onesk": np.ones((128, 128), np.float32),
        "jcon": np.ascontiguousarray(jc),
        "cvec": np.ascontiguousarray(cv),
    }


def kernel(**inputs):
    if "nc" not in _CACHE:
        _CACHE["nc"] = _build()
    nc = _CACHE["nc"]

    x = np.asarray(inputs["x"], np.float32)
    shared = _host_layouts(inputs)
    in_maps = []
    for core in range(NCORES):
        m = dict(shared)
        m["x"] = np.ascontiguousarray(x[core * BS:(core + 1) * BS])
        in_maps.append(m)

    trace = bool(int(os.environ.get("BASS_KERNEL_TRACE", "0")))
    kw = {}
    if trace:
        from trn_agent_boot.trn_boot import _ntff_profile_via_ctypes
        import antenv.axon_hooks as ah
        ah.set_axon_ntff_profile_hook(
            _ntff_profile_via_ctypes("/opt/axon/libaxon_pjrt.so"))
        import tempfile
        base = os.environ.get("BASS_KERNEL_TRACE_DIR", "/tmp/adaptconv_trace")
        os.makedirs(base, exist_ok=True)
        kw = dict(trace=True, tmpdir=tempfile.mkdtemp(dir=base))

    res = run_bass_kernel_spmd(nc, in_maps, core_ids=list(range(NCORES)), **kw)
    _CACHE["last_exec_time_ns"] = res.exec_time_ns

    _CACHE["dbg"] = [res.results[i].get("dbg") for i in range(NCORES)]
    out = np.concatenate(
        [res.results[i]["out"].reshape(BS, C, H, W) for i in range(NCORES)],
        axis=0)
    return out


# revision 16
# speedup vs baseline: 1.4353x; 1.2178x over previous
"""AdaptConv2d Trainium2 kernel: 8-core data-parallel, gate-driven sparse conv.

Computes, per sample b:
  layer_bit = (LSTM-gate pre-activation > 0)
  if layer_bit:  channel mask m_c = (channel-gate fc pre-activation > 0)
                 out[c] = conv3x3(x)[c] if m_c else x[c]
  else:          out = x

Device strategy per core (4 samples):
  - x loaded into zero-padded (58x58) SBUF images, f32r (tf32) typed for the
    TensorEngine fast path; bits are untouched so pass-through output is exact.
  - Layer gate (GAP + 1x1-conv + single-step LSTM + fc) in true fp32 for all 4
    samples, branch-free.
  - Per sample, a 0/1-trip For_i (trip = layer bit) guards the heavy work:
    stride-2 channel-gate conv (tf32 matmuls, fp32 accum), fp32 fc -> binary
    mask, prefix-sum -> one-hot selection matrix S, PE-side weight gather
    (W^T @ S), compact conv over only ceil(n_active/128) 128-channel blocks
    (inner 0/1-trip For_i for the second block), and an indirect row-scatter
    of conv rows into the output (out-of-bounds pad rows silently dropped).
  - Unconditional default write out = x covers inactive channels/samples.
"""

import os
import sys
import types

sys.path.insert(0, "/opt/trn_rl_repo")

import numpy as np

# antenv.axon_hooks is missing from this image; inject a minimal stand-in so
# run_bass_kernel_spmd's trace path imports cleanly (used only when tracing).
try:
    import antenv  # noqa: F401

    if "antenv.axon_hooks" not in sys.modules:
        _m = types.ModuleType("antenv.axon_hooks")
        _h = [None]
        _m.set_axon_ntff_profile_hook = lambda hook: _h.__setitem__(0, hook)
        _m.get_axon_ntff_profile_hook = lambda: _h[0]
        sys.modules["antenv.axon_hooks"] = _m
        antenv.axon_hooks = _m
except Exception:
    pass

import concourse.bass as bass
import concourse.mybir as mybir
from concourse import bacc
from concourse.expressions import smin
from concourse.tile import TileContext
from concourse.bass_utils import run_bass_kernel_spmd

F32 = mybir.dt.float32
F32R = mybir.dt.float32r
I32 = mybir.dt.int32
AF = mybir.ActivationFunctionType
ALU = mybir.AluOpType

B, C, H, W = 32, 256, 56, 56
NCORES = 8
BS = B // NCORES          # samples per core
HW = H * W                # 3136
PH, PW = H + 2, W + 2     # 58x58 padded image
PHW = PH * PW             # 3364
XT_COLS = PHW + 4         # tail pad: edge-tap reads run 2 past the image
LSTM_H = 10
ENGINES = list(mybir.ALL_ENGINES)

# main-conv spatial chunking: 7 chunks x 8 valid rows; each chunk is a
# contiguous 464-wide span of the padded image (includes L/R pad cols, whose
# outputs are junk and excluded at extraction time)
NCHUNK = 7
CH_ROWS = 8
CH_N = CH_ROWS * PH       # 464

# channel-gate conv: 27x27 valid outputs, row-chunks of 14/13, 28 cols (28th
# col junk so the fp32r moving operand has an even innermost count)
G_CHUNKS = ((0, 14), (14, 27))
G_COLS = 28

_CACHE = {}


def _build():
    nc = bacc.Bacc(None, target_bir_lowering=False)

    xp = nc.declare_dram_parameter("x", [BS, C, H, W], F32, isOutput=False)
    outp = nc.declare_dram_parameter("out", [BS, C, HW], F32, isOutput=True)
    wnat = nc.declare_dram_parameter("wnat", [2, 128, 9 * C], F32R, isOutput=False)
    cgw = nc.declare_dram_parameter("cgw", [2, 128, 9 * C], F32R, isOutput=False)
    fcwt = nc.declare_dram_parameter("fcwt", [2, 128, C], F32, isOutput=False)
    lgwt = nc.declare_dram_parameter("lgwt", [2, 128, LSTM_H], F32, isOutput=False)
    wiht = nc.declare_dram_parameter("wiht", [LSTM_H + 1, 4 * LSTM_H], F32, isOutput=False)
    lgfc = nc.declare_dram_parameter("lgfc", [1, LSTM_H], F32, isOutput=False)
    cgb = nc.declare_dram_parameter("cgb", [128, 2], F32, isOutput=False)
    fcb = nc.declare_dram_parameter("fcb", [128, 2], F32, isOutput=False)
    lgb = nc.declare_dram_parameter("lgb", [LSTM_H, 1], F32, isOutput=False)
    lfb = nc.declare_dram_parameter("lfb", [1, 1], F32, isOutput=False)
    ucon = nc.declare_dram_parameter("ucon", [128, 128], F32, isOutput=False)
    onesk = nc.declare_dram_parameter("onesk", [128, 128], F32, isOutput=False)
    jcon = nc.declare_dram_parameter("jcon", [128, 2 * 128], F32, isOutput=False)
    cvec = nc.declare_dram_parameter("cvec", [128, 2], F32, isOutput=False)
    dbg = nc.declare_dram_parameter("dbg", [128, 16], F32, isOutput=True)

    with TileContext(nc) as tc:
        with tc.tile_pool(name="sbuf", bufs=1) as pc, \
             tc.tile_pool(name="work", bufs=1) as pw, \
             tc.tile_pool(name="psum", bufs=1, space="PSUM") as pp:

            # ---- constants / weights resident in SBUF ----
            ucon_t = pc.tile([128, 128], F32, tag="ucon")
            nc.sync.dma_start(out=ucon_t[:], in_=ucon[:])
            ones_t = pc.tile([128, 128], F32, tag="ones")
            nc.sync.dma_start(out=ones_t[:], in_=onesk[:])
            j_t = pc.tile([128, 256], F32, tag="jcon")
            nc.sync.dma_start(out=j_t[:], in_=jcon[:])
            cvec_t = pc.tile([128, 2], F32, tag="cvec")
            nc.sync.dma_start(out=cvec_t[:], in_=cvec[:])
            lgwt_t = pc.tile([128, 2 * LSTM_H], F32, tag="lgwt")
            nc.sync.dma_start(out=lgwt_t[:, 0:LSTM_H], in_=lgwt[0])
            nc.sync.dma_start(out=lgwt_t[:, LSTM_H:2 * LSTM_H], in_=lgwt[1])
            wiht_t = pc.tile([LSTM_H + 1, 4 * LSTM_H], F32, tag="wiht")
            nc.sync.dma_start(out=wiht_t[:], in_=wiht[:])
            lgb_t = pc.tile([LSTM_H, 1], F32, tag="lgb")
            nc.sync.dma_start(out=lgb_t[:], in_=lgb[:])
            lgfc_t = pc.tile([1, LSTM_H], F32, tag="lgfc")
            nc.sync.dma_start(out=lgfc_t[:], in_=lgfc[:])
            lfb_t = pc.tile([1, 1], F32, tag="lfb")
            nc.sync.dma_start(out=lfb_t[:], in_=lfb[:])

            zeros1 = pc.tile([1, 1], F32, tag="z1")
            nc.vector.memset(zeros1[:], 0.0)
            zeros128 = pc.tile([128, 1], F32, tag="z128")
            nc.vector.memset(zeros128[:], 0.0)

            out_rows = outp[:].rearrange("a c n -> (a c) n")
            g1 = pc.tile([128, 2 * BS], F32, tag="g1")   # GAP sums, col kb*BS+si
            htile = pc.tile([LSTM_H + 1, BS], F32, tag="htile")
            nc.sync.dma_start(out=htile[LSTM_H:LSTM_H + 1, :],
                              in_=onesk[0:1, 0:BS])
            dbg_t = pc.tile([128, 16], F32, tag="dbg")
            nc.vector.memset(dbg_t[:], 0.0)

            # conv/gate weights (stream in behind the first x tiles)
            wnat_t = pc.tile([128, 2 * 9 * C], F32R, tag="wnat")
            cgw_t = pc.tile([128, 2 * 9 * C], F32R, tag="cgw")
            fcwt_t = pc.tile([128, 2 * C], F32, tag="fcwt")
            cgb_t = pc.tile([128, 2], F32, tag="cgb")
            fcb_s = pc.tile([128, 2], F32, tag="fcbs")

            l_vals = []
            for si in range(BS):
                # ---- stream x: exact pass-through + exact GAP ----
                xus = []
                for kb in range(2):
                    xu = pw.tile([128, HW], F32, tag="xu", bufs=4)
                    nc.sync.dma_start(out=xu[:],
                                      in_=xp[si, kb * 128:(kb + 1) * 128]
                                      .rearrange("p a b -> p (a b)"))
                    xus.append(xu)
                if si == 0:
                    nc.sync.dma_start(out=wnat_t[:, 0:9 * C], in_=wnat[0])
                    nc.sync.dma_start(out=wnat_t[:, 9 * C:2 * 9 * C], in_=wnat[1])
                    nc.sync.dma_start(out=cgw_t[:, 0:9 * C], in_=cgw[0])
                    nc.sync.dma_start(out=cgw_t[:, 9 * C:2 * 9 * C], in_=cgw[1])
                    nc.sync.dma_start(out=fcwt_t[:, 0:C], in_=fcwt[0])
                    nc.sync.dma_start(out=fcwt_t[:, C:2 * C], in_=fcwt[1])
                    nc.sync.dma_start(out=cgb_t[:], in_=cgb[:])
                    nc.sync.dma_start(out=fcb_s[:], in_=fcb[:])
                    # mask threshold works on raw GAP sums: scale the bias by
                    # the 27x27 GAP count instead of dividing the sums
                    nc.vector.tensor_scalar_mul(fcb_s[:], fcb_s[:], 729.0)
                for kb in range(2):
                    col = kb * BS + si
                    if kb == 0:
                        nc.vector.tensor_reduce(
                            out=g1[:, col:col + 1], in_=xus[kb][:],
                            axis=mybir.AxisListType.X, op=ALU.add)
                    else:
                        gsc = pw.tile([128, HW], F32, tag="gapscr")
                        nc.scalar.activation(gsc[:], xus[kb][:], AF.Copy,
                                             accum_out=g1[:, col:col + 1])
                    nc.sync.dma_start(out=outp[si, kb * 128:(kb + 1) * 128],
                                      in_=xus[kb][:])

                # ---- layer gate (true fp32) for this sample ----
                ph = pp.tile([LSTM_H, 1], F32, tag="sel")
                for kb in range(2):
                    nc.tensor.matmul(
                        ph[:], lgwt_t[:, kb * LSTM_H:(kb + 1) * LSTM_H],
                        g1[:, kb * BS + si:kb * BS + si + 1],
                        start=(kb == 0), stop=(kb == 1))
                nc.scalar.activation(htile[0:LSTM_H, si:si + 1], ph[:], AF.Relu,
                                     bias=lgb_t[:, 0:1], scale=1.0 / HW)
                pg = pp.tile([1, 4 * LSTM_H], F32, tag="sel")
                nc.tensor.matmul(pg[:], htile[:, si:si + 1], wiht_t[:],
                                 start=True, stop=True)
                lw = pw.tile([1, 4 * LSTM_H], F32, tag="lw", bufs=2)
                nc.scalar.activation(lw[:, 0:LSTM_H], pg[:, 0:LSTM_H], AF.Sigmoid)
                nc.scalar.activation(lw[:, 3 * LSTM_H:4 * LSTM_H],
                                     pg[:, 3 * LSTM_H:4 * LSTM_H], AF.Sigmoid)
                nc.scalar.activation(lw[:, 2 * LSTM_H:3 * LSTM_H],
                                     pg[:, 2 * LSTM_H:3 * LSTM_H], AF.Tanh)
                cb_t = pw.tile([1, LSTM_H], F32, tag="cbuf", bufs=2)
                nc.vector.tensor_tensor(out=cb_t[:], in0=lw[:, 0:LSTM_H],
                                        in1=lw[:, 2 * LSTM_H:3 * LSTM_H],
                                        op=ALU.mult)
                eb_t = pw.tile([1, LSTM_H], F32, tag="ebuf", bufs=2)
                nc.scalar.activation(eb_t[:], cb_t[:], AF.Tanh)
                hs_t = pw.tile([1, LSTM_H], F32, tag="hsb", bufs=2)
                nc.vector.tensor_tensor(out=hs_t[:],
                                        in0=lw[:, 3 * LSTM_H:4 * LSTM_H],
                                        in1=eb_t[:], op=ALU.mult)
                pr_t = pw.tile([1, LSTM_H], F32, tag="prod", bufs=2)
                nc.vector.tensor_tensor(out=pr_t[:], in0=hs_t[:], in1=lgfc_t[:],
                                        op=ALU.mult)
                lpre = pw.tile([1, 1], F32, tag="lpre", bufs=2)
                nc.vector.tensor_reduce(out=lpre[:], in_=pr_t[:],
                                        axis=mybir.AxisListType.X, op=ALU.add)
                l_bin = pw.tile([1, 1], F32, tag="lbin", bufs=2)
                nc.vector.scalar_tensor_tensor(out=l_bin[:], in0=lpre[:],
                                               scalar=lfb_t[:, 0:1],
                                               in1=zeros1[:],
                                               op0=ALU.add, op1=ALU.is_gt)
                nc.vector.tensor_copy(out=dbg_t[0:1, si:si + 1], in_=lpre[:])
                nc.vector.tensor_copy(out=dbg_t[0:1, 4 + si:5 + si], in_=l_bin[:])
                l_i32 = pw.tile([1, 1], I32, tag="li32", bufs=2)
                nc.vector.tensor_copy(out=l_i32[:], in_=l_bin[:])
                l_vals.append(nc.values_load(l_i32[0:1, 0:1], engines=ENGINES,
                                             min_val=0, max_val=1,
                                             skip_runtime_bounds_check=True))

                # ---- gated heavy path for this sample ----
                with tc.For_i(0, l_vals[si], 1):
                    # re-load x as f32r: flat tiles for the stride-2 gate conv,
                    # zero-padded images for the main conv
                    xrs, xts = [], []
                    for kb in range(2):
                        xr = pw.tile([128, HW + 4], F32R, tag=f"xr{kb}")
                        nc.gpsimd.dma_start(out=xr[:, 0:HW],
                                            in_=xp[si, kb * 128:(kb + 1) * 128]
                                            .rearrange("p a b -> p (a b)"))
                        nc.vector.memset(xr[:, HW:HW + 4].bitcast(F32), 0.0)
                        xrs.append(xr)
                    for kb in range(2):
                        xt = pw.tile([128, XT_COLS], F32R, tag=f"xpad{kb}")
                        xv = xt[:, 0:PHW].rearrange("p (h w) -> p h w", h=PH)
                        nc.vector.memset(xv[:, 0:1, :].bitcast(F32), 0.0)
                        nc.vector.memset(xv[:, PH - 1:PH, :].bitcast(F32), 0.0)
                        nc.vector.memset(xv[:, :, 0:1].bitcast(F32), 0.0)
                        nc.vector.memset(xv[:, :, PW - 1:PW].bitcast(F32), 0.0)
                        nc.vector.memset(xt[:, PHW:XT_COLS].bitcast(F32), 0.0)
                        src = xrs[kb][:, 0:HW].bitcast(F32).rearrange(
                            "p (a b) -> p a b", a=H)
                        if kb == 0:
                            nc.vector.tensor_copy(
                                out=xv[:, 1:PH - 1, 1:PW - 1], in_=src)
                        else:
                            nc.scalar.activation(
                                xv[:, 1:PH - 1, 1:PW - 1], src, AF.Copy)
                        xts.append(xt)

                    # -- channel-gate conv (stride-2 valid 3x3) + GAP --
                    g2 = pw.tile([128, 2], F32, tag="g2")
                    for cb in range(2):
                        accs = []
                        for (r0, r1) in G_CHUNKS:
                            rows = r1 - r0
                            pgc = pp.tile([128, rows * G_COLS], F32, tag="conv",
                                          bufs=7)
                            first = True
                            for tap in range(9):
                                dy, dx = tap // 3, tap % 3
                                for kb in range(2):
                                    off = (2 * r0 + dy) * W + dx
                                    rhs = xrs[kb][:, off:off + 112 * rows] \
                                        .rearrange("p (a b) -> p a b", b=112) \
                                        [:, :, 0:2 * G_COLS:2]
                                    nc.tensor.matmul(
                                        pgc[:],
                                        cgw_t[:, kb * 9 * C + tap * C + cb * 128:
                                              kb * 9 * C + tap * C + cb * 128 + 128],
                                        rhs,
                                        start=first, stop=(tap == 8 and kb == 1))
                                    first = False
                            scr = pw.tile([128, 14 * G_COLS], F32, tag="gscr",
                                          bufs=2)
                            acc = pw.tile([128, 1], F32, tag=f"gacc{len(accs)}")
                            pv = pgc[:].rearrange("p (r c) -> p r c", c=G_COLS)
                            sv = scr[:].rearrange("p (r c) -> p r c", c=G_COLS)
                            nc.scalar.activation(sv[:, 0:rows, 0:27],
                                                 pv[:, :, 0:27], AF.Relu,
                                                 bias=cgb_t[:, cb:cb + 1],
                                                 accum_out=acc[:])
                            accs.append(acc)
                        nc.vector.tensor_tensor(out=g2[:, cb:cb + 1],
                                                in0=accs[0][:], in1=accs[1][:],
                                                op=ALU.add)

                    # -- fc -> binary mask per channel --
                    m_t = pw.tile([128, 2], F32, tag="mt")
                    for cbm in range(2):
                        pf = pp.tile([128, 1], F32, tag="sel")
                        for kb in range(2):
                            nc.tensor.matmul(
                                pf[:],
                                fcwt_t[:, kb * C + cbm * 128:kb * C + cbm * 128 + 128],
                                g2[:, kb:kb + 1],
                                start=(kb == 0), stop=(kb == 1))
                        nc.vector.scalar_tensor_tensor(
                            out=m_t[:, cbm:cbm + 1], in0=pf[:],
                            scalar=fcb_s[:, cbm:cbm + 1], in1=zeros128[:],
                            op0=ALU.add, op1=ALU.is_gt)

                    # -- n = #active, exclusive prefix, one-hot S, row indices --
                    pn = pp.tile([1, 1], F32, tag="sel")
                    for cb in range(2):
                        nc.tensor.matmul(pn[:], ones_t[:, 0:1], m_t[:, cb:cb + 1],
                                         start=(cb == 0), stop=(cb == 1))
                    n_sb = pw.tile([1, 1], F32, tag="nsb")
                    nc.scalar.activation(n_sb[:], pn[:], AF.Copy)
                    n_i32 = pw.tile([1, 2], I32, tag="ni32")
                    nc.vector.tensor_copy(out=n_i32[:, 0:1], in_=n_sb[:])
                    n2_sb = pw.tile([1, 1], F32, tag="n2sb")
                    nc.vector.tensor_scalar_sub(n2_sb[:], n_sb[:], 128.0)
                    nc.vector.tensor_copy(out=n_i32[:, 1:2], in_=n2_sb[:])
                    n_bc = pw.tile([128, 1], F32, tag="nbc")
                    nc.gpsimd.partition_broadcast(n_bc[:], n_sb[:])

                    pos_sb = pw.tile([128, 2], F32, tag="pos")
                    pp0 = pp.tile([128, 1], F32, tag="sel")
                    nc.tensor.matmul(pp0[:], ucon_t[:], m_t[:, 0:1],
                                     start=True, stop=True)
                    nc.scalar.activation(pos_sb[:, 0:1], pp0[:], AF.Copy)
                    pp1 = pp.tile([128, 1], F32, tag="sel")
                    nc.tensor.matmul(pp1[:], ones_t[:], m_t[:, 0:1],
                                     start=True, stop=False)
                    nc.tensor.matmul(pp1[:], ucon_t[:], m_t[:, 1:2],
                                     start=False, stop=True)
                    nc.scalar.activation(pos_sb[:, 1:2], pp1[:], AF.Copy)

                    s_ts = []
                    for cb in range(2):
                        s_t = pw.tile([128, 256], F32, tag=f"s{cb}")
                        nc.vector.tensor_scalar(
                            out=s_t[:].bitcast(F32R), in0=j_t[:],
                            scalar1=pos_sb[:, cb:cb + 1],
                            scalar2=None, op0=ALU.is_equal)
                        nc.vector.tensor_scalar(
                            out=s_t[:].bitcast(F32R), in0=s_t[:],
                            scalar1=m_t[:, cb:cb + 1], scalar2=None, op0=ALU.mult)
                        s_ts.append(s_t)

                    idx_i32 = pw.tile([128, 2], I32, tag="idxi")
                    for j in range(2):
                        pi = pp.tile([128, 1], F32, tag="sel")
                        for cb in range(2):
                            nc.tensor.matmul(pi[:],
                                             s_ts[cb][:, j * 128:(j + 1) * 128],
                                             cvec_t[:, cb:cb + 1],
                                             start=(cb == 0), stop=(cb == 1))
                        cmp = pw.tile([128, 1], F32, tag="cmp")
                        nc.vector.tensor_scalar(
                            out=cmp[:], in0=cvec_t[:, j:j + 1],
                            scalar1=n_bc[:, 0:1], scalar2=None, op0=ALU.is_ge)
                        idxf = pw.tile([128, 1], F32, tag="idxf")
                        nc.vector.scalar_tensor_tensor(
                            out=idxf[:], in0=cmp[:], scalar=4096.0, in1=pi[:],
                            op0=ALU.mult, op1=ALU.add)
                        nc.vector.tensor_scalar(
                            out=idxf[:], in0=idxf[:], scalar1=float(si * C),
                            scalar2=None, op0=ALU.add)
                        nc.vector.tensor_copy(out=idx_i32[:, j:j + 1], in_=idxf[:])

                    n_val = nc.values_load(n_i32[0:1, 0:1], engines=ENGINES,
                                           min_val=0, max_val=256,
                                           skip_runtime_bounds_check=True)
                    n2_val = nc.values_load(n_i32[0:1, 1:2], engines=ENGINES,
                                            min_val=-256, max_val=128,
                                            skip_runtime_bounds_check=True)

                    # -- weight gather interleaved with block-0 compact conv --
                    # selw[:, wi*256 + p] = W[idx_p, cin] for both p-blocks
                    selw = pw.tile([128, 18 * 256], F32R, tag="selw")
                    banks = [pp.tile([128, CH_N], F32, tag="conv", bufs=7,
                                     name=f"bank{_k}")
                             for _k in range(NCHUNK)]

                    def emit_sel(wi):
                        tap, kb = wi // 2, wi % 2
                        ps = pp.tile([128, 256], F32, tag="sel", name=f"ps{wi}")
                        for cb in range(2):
                            nc.tensor.matmul(
                                ps[:],
                                wnat_t[:, cb * 9 * C + tap * C + kb * 128:
                                       cb * 9 * C + tap * C + kb * 128 + 128],
                                s_ts[cb][:].bitcast(F32R),
                                start=(cb == 0), stop=(cb == 1))
                        nc.vector.tensor_copy(
                            out=selw[:, wi * 256:(wi + 1) * 256], in_=ps[:])

                    def emit_conv(wi, j):
                        tap, kb = wi // 2, wi % 2
                        dy, dx = tap // 3, tap % 3
                        xt = xts[kb]
                        for k in range(NCHUNK):
                            off = (CH_ROWS * k + dy) * PH + dx
                            nc.tensor.matmul(
                                banks[k][:],
                                selw[:, wi * 256 + j * 128:
                                     wi * 256 + j * 128 + 128],
                                xt[:, off:off + CH_N],
                                start=(wi == 0), stop=(wi == 17))

                    def emit_out(j):
                        stg = pw.tile([128, HW], F32, tag="stg", name=f"stg{j}")
                        for k in range(NCHUNK):
                            bv = banks[k][:].rearrange("p (r c) -> p r c", c=PH)
                            sv = stg[:].rearrange("p (r c) -> p r c", c=W)
                            nc.scalar.activation(
                                sv[:, k * CH_ROWS:(k + 1) * CH_ROWS, :],
                                bv[:, :, 0:W], AF.Copy)
                        nc.gpsimd.indirect_dma_start(
                            out=out_rows,
                            out_offset=bass.IndirectOffsetOnAxis(
                                ap=idx_i32[:, j:j + 1], axis=0),
                            in_=stg[:], in_offset=None,
                            bounds_check=BS * C - 1, oob_is_err=False)

                    emit_sel(0)
                    for wi in range(18):
                        if wi < 17:
                            emit_sel(wi + 1)
                        emit_conv(wi, 0)
                    emit_out(0)
                    with tc.For_i(0, smin(n2_val, 1), 1):
                        banks = [pp.tile([128, CH_N], F32, tag="conv", bufs=7,
                                         name=f"bankb{_k}")
                                 for _k in range(NCHUNK)]
                        for wi in range(18):
                            emit_conv(wi, 1)
                        emit_out(1)

            nc.sync.dma_start(out=dbg[:], in_=dbg_t[:])

    nc.compile()
    return nc


def _host_layouts(inputs):
    conv_w = np.asarray(inputs["conv_w"], np.float32)
    cg_conv_w = np.asarray(inputs["cg_conv_w"], np.float32)
    cg_fc_w = np.asarray(inputs["cg_fc_w"], np.float32)
    lg_conv_w = np.asarray(inputs["lg_conv_w"], np.float32)
    w_ih = np.asarray(inputs["lstm_w_ih"], np.float32)

    # wnat[cb][cout, tap*256+cin] = conv_w[cb*128+cout, cin, dy, dx]
    wn = conv_w.transpose(0, 2, 3, 1).reshape(C, 9 * C)
    wnat = np.ascontiguousarray(wn.reshape(2, 128, 9 * C))
    # cgw[kb][cin, tap*256+cout] = cg_conv_w[cout, kb*128+cin, dy, dx]
    cg = cg_conv_w.transpose(1, 2, 3, 0).reshape(C, 9 * C)
    cgw = np.ascontiguousarray(cg.reshape(2, 128, 9 * C))
    # fcwt[kb][k, c] = cg_fc_w[c, kb*128+k]
    fcwt = np.ascontiguousarray(cg_fc_w.T.reshape(2, 128, C))
    # lgwt[kb][k, m] = lg_conv_w[m, kb*128+k]
    lgwt = np.ascontiguousarray(
        lg_conv_w.reshape(LSTM_H, C).T.reshape(2, 128, LSTM_H))
    wiht = np.concatenate(
        [w_ih.T, (np.asarray(inputs["lstm_b_ih"], np.float32)
                  + np.asarray(inputs["lstm_b_hh"], np.float32))[None, :]],
        axis=0)
    wiht = np.ascontiguousarray(wiht)

    cgb = np.ascontiguousarray(
        np.asarray(inputs["cg_conv_b"], np.float32).reshape(2, 128).T)
    fcb = np.ascontiguousarray(
        np.asarray(inputs["cg_fc_b"], np.float32).reshape(2, 128).T)

    u = np.triu(np.ones((128, 128), np.float32), k=1)
    jc = np.tile(np.arange(256, dtype=np.float32)[None, :], (128, 1))
    cv = np.stack([np.arange(128, dtype=np.float32),
                   np.arange(128, 256, dtype=np.float32)], axis=1)

    return {
        "wnat": wnat, "cgw": cgw, "fcwt": fcwt, "lgwt": lgwt, "wiht": wiht,
        "lgfc": np.ascontiguousarray(
            np.asarray(inputs["lg_fc_w"], np.float32).reshape(1, LSTM_H)),
        "cgb": cgb, "fcb": fcb,
        "lgb": np.ascontiguousarray(
            np.asarray(inputs["lg_conv_b"], np.float32).reshape(LSTM_H, 1)),
        "lfb": np.ascontiguousarray(
            np.asarray(inputs["lg_fc_b"], np.float32).reshape(1, 1)),
        "ucon": np.ascontiguousarray(u),
        "onesk": np.ones((128, 128), np.float32),
        "jcon": np.ascontiguousarray(jc),
        "cvec": np.ascontiguousarray(cv),
    }


def kernel(**inputs):
    if "nc" not in _CACHE:
        _CACHE["nc"] = _build()
    nc = _CACHE["nc"]

    x = np.asarray(inputs["x"], np.float32)
    shared = _host_layouts(inputs)
    in_maps = []
    for core in range(NCORES):
        m = dict(shared)
        m["x"] = np.ascontiguousarray(x[core * BS:(core + 1) * BS])
        in_maps.append(m)

    trace = bool(int(os.environ.get("BASS_KERNEL_TRACE", "0")))
    kw = {}
    if trace:
        from trn_agent_boot.trn_boot import _ntff_profile_via_ctypes
        import antenv.axon_hooks as ah
        ah.set_axon_ntff_profile_hook(
            _ntff_profile_via_ctypes("/opt/axon/libaxon_pjrt.so"))
        import tempfile
        base = os.environ.get("BASS_KERNEL_TRACE_DIR", "/tmp/adaptconv_trace")
        os.makedirs(base, exist_ok=True)
        kw = dict(trace=True, tmpdir=tempfile.mkdtemp(dir=base))

    res = run_bass_kernel_spmd(nc, in_maps, core_ids=list(range(NCORES)), **kw)
    _CACHE["last_exec_time_ns"] = res.exec_time_ns

    _CACHE["dbg"] = [res.results[i].get("dbg") for i in range(NCORES)]
    out = np.concatenate(
        [res.results[i]["out"].reshape(BS, C, H, W) for i in range(NCORES)],
        axis=0)
    return out


# revision 21
# speedup vs baseline: 1.4741x; 1.0271x over previous
"""AdaptConv2d Trainium2 kernel: 8-core data-parallel, gate-driven sparse conv.

Computes, per sample b:
  layer_bit = (LSTM-gate pre-activation > 0)
  if layer_bit:  channel mask m_c = (channel-gate fc pre-activation > 0)
                 out[c] = conv3x3(x)[c] if m_c else x[c]
  else:          out = x

Device strategy per core (4 samples):
  - x loaded into zero-padded (58x58) SBUF images, f32r (tf32) typed for the
    TensorEngine fast path; bits are untouched so pass-through output is exact.
  - Layer gate (GAP + 1x1-conv + single-step LSTM + fc) in true fp32 for all 4
    samples, branch-free.
  - Per sample, a 0/1-trip For_i (trip = layer bit) guards the heavy work:
    stride-2 channel-gate conv (tf32 matmuls, fp32 accum), fp32 fc -> binary
    mask, prefix-sum -> one-hot selection matrix S, PE-side weight gather
    (W^T @ S), compact conv over only ceil(n_active/128) 128-channel blocks
    (inner 0/1-trip For_i for the second block), and an indirect row-scatter
    of conv rows into the output (out-of-bounds pad rows silently dropped).
  - Unconditional default write out = x covers inactive channels/samples.
"""

import os
import sys
import types

sys.path.insert(0, "/opt/trn_rl_repo")

import numpy as np

# antenv.axon_hooks is missing from this image; inject a minimal stand-in so
# run_bass_kernel_spmd's trace path imports cleanly (used only when tracing).
try:
    import antenv  # noqa: F401

    if "antenv.axon_hooks" not in sys.modules:
        _m = types.ModuleType("antenv.axon_hooks")
        _h = [None]
        _m.set_axon_ntff_profile_hook = lambda hook: _h.__setitem__(0, hook)
        _m.get_axon_ntff_profile_hook = lambda: _h[0]
        sys.modules["antenv.axon_hooks"] = _m
        antenv.axon_hooks = _m
except Exception:
    pass

import concourse.bass as bass
import concourse.mybir as mybir
from concourse import bacc
from concourse.expressions import smin
from concourse.tile import TileContext
from concourse.bass_utils import run_bass_kernel_spmd

F32 = mybir.dt.float32
F32R = mybir.dt.float32r
I32 = mybir.dt.int32
AF = mybir.ActivationFunctionType
ALU = mybir.AluOpType

B, C, H, W = 32, 256, 56, 56
NCORES = 8
BS = B // NCORES          # samples per core
HW = H * W                # 3136
PH, PW = H + 2, W + 2     # 58x58 padded image
PHW = PH * PW             # 3364
XT_COLS = PHW + 4         # tail pad: edge-tap reads run 2 past the image
LSTM_H = 10
ENGINES = list(mybir.ALL_ENGINES)

# main-conv spatial chunking: 7 chunks x 8 valid rows; each chunk is a
# contiguous 464-wide span of the padded image (includes L/R pad cols, whose
# outputs are junk and excluded at extraction time)
NCHUNK = 7
CH_ROWS = 8
CH_N = CH_ROWS * PH       # 464

# channel-gate conv: 27x27 valid outputs, row-chunks of 14/13, 28 cols (28th
# col junk so the fp32r moving operand has an even innermost count)
G_CHUNKS = ((0, 14), (14, 27))
G_COLS = 28

_CACHE = {}


def _build():
    nc = bacc.Bacc(None, target_bir_lowering=False)

    xp = nc.declare_dram_parameter("x", [BS, C, H, W], F32, isOutput=False)
    outp = nc.declare_dram_parameter("out", [BS, C, HW], F32, isOutput=True)
    wnat = nc.declare_dram_parameter("wnat", [2, 128, 9 * C], F32R, isOutput=False)
    cgw = nc.declare_dram_parameter("cgw", [2, 128, 9 * C], F32R, isOutput=False)
    fcwt = nc.declare_dram_parameter("fcwt", [2, 128, C], F32, isOutput=False)
    lgwt = nc.declare_dram_parameter("lgwt", [2, 128, LSTM_H], F32, isOutput=False)
    wiht = nc.declare_dram_parameter("wiht", [LSTM_H + 1, 4 * LSTM_H], F32, isOutput=False)
    lgfc = nc.declare_dram_parameter("lgfc", [1, LSTM_H], F32, isOutput=False)
    cgb = nc.declare_dram_parameter("cgb", [128, 2], F32, isOutput=False)
    fcb = nc.declare_dram_parameter("fcb", [128, 2], F32, isOutput=False)
    lgb = nc.declare_dram_parameter("lgb", [LSTM_H, 1], F32, isOutput=False)
    lfb = nc.declare_dram_parameter("lfb", [1, 1], F32, isOutput=False)
    ucon = nc.declare_dram_parameter("ucon", [128, 128], F32, isOutput=False)
    onesk = nc.declare_dram_parameter("onesk", [128, 128], F32, isOutput=False)
    jcon = nc.declare_dram_parameter("jcon", [128, 2 * 128], F32, isOutput=False)
    cvec = nc.declare_dram_parameter("cvec", [128, 2], F32, isOutput=False)
    dbg = nc.declare_dram_parameter("dbg", [128, 16], F32, isOutput=True)

    with TileContext(nc) as tc:
        with tc.tile_pool(name="sbuf", bufs=1) as pc, \
             tc.tile_pool(name="work", bufs=1) as pw, \
             tc.tile_pool(name="psum", bufs=1, space="PSUM") as pp:

            # ---- constants / weights resident in SBUF ----
            ucon_t = pc.tile([128, 128], F32, tag="ucon")
            nc.sync.dma_start(out=ucon_t[:], in_=ucon[:])
            ones_t = pc.tile([128, 128], F32, tag="ones")
            nc.sync.dma_start(out=ones_t[:], in_=onesk[:])
            j_t = pc.tile([128, 256], F32, tag="jcon")
            nc.sync.dma_start(out=j_t[:], in_=jcon[:])
            cvec_t = pc.tile([128, 2], F32, tag="cvec")
            nc.sync.dma_start(out=cvec_t[:], in_=cvec[:])
            lgwt_t = pc.tile([128, 2 * LSTM_H], F32, tag="lgwt")
            nc.sync.dma_start(out=lgwt_t[:, 0:LSTM_H], in_=lgwt[0])
            nc.sync.dma_start(out=lgwt_t[:, LSTM_H:2 * LSTM_H], in_=lgwt[1])
            wiht_t = pc.tile([LSTM_H + 1, 4 * LSTM_H], F32, tag="wiht")
            nc.sync.dma_start(out=wiht_t[:], in_=wiht[:])
            lgb_t = pc.tile([LSTM_H, 1], F32, tag="lgb")
            nc.sync.dma_start(out=lgb_t[:], in_=lgb[:])
            lgfc_t = pc.tile([1, LSTM_H], F32, tag="lgfc")
            nc.sync.dma_start(out=lgfc_t[:], in_=lgfc[:])
            lfb_t = pc.tile([1, 1], F32, tag="lfb")
            nc.sync.dma_start(out=lfb_t[:], in_=lfb[:])

            zeros1 = pc.tile([1, 1], F32, tag="z1")
            nc.vector.memset(zeros1[:], 0.0)
            zeros128 = pc.tile([128, 1], F32, tag="z128")
            nc.vector.memset(zeros128[:], 0.0)

            out_rows = outp[:].rearrange("a c n -> (a c) n")
            g1 = pc.tile([128, 2 * BS], F32, tag="g1")   # GAP sums, col kb*BS+si
            htile = pc.tile([LSTM_H + 1, BS], F32, tag="htile")
            nc.sync.dma_start(out=htile[LSTM_H:LSTM_H + 1, :],
                              in_=onesk[0:1, 0:BS])
            dbg_t = pc.tile([128, 16], F32, tag="dbg")
            nc.vector.memset(dbg_t[:], 0.0)

            # conv/gate weights (stream in behind the first x tiles)
            wnat_t = pc.tile([128, 2 * 9 * C], F32R, tag="wnat")
            cgw_t = pc.tile([128, 2 * 9 * C], F32R, tag="cgw")
            fcwt_t = pc.tile([128, 2 * C], F32, tag="fcwt")
            cgb_t = pc.tile([128, 2], F32, tag="cgb")
            fcb_s = pc.tile([128, 2], F32, tag="fcbs")

            warm_a = pc.tile([128, 128], F32, tag="warma")
            nc.sync.dma_start(out=warm_a[:], in_=onesk[:])
            warm_b = pc.tile([128, 128], F32, tag="warmb")
            nc.sync.dma_start(out=warm_b[:], in_=onesk[:])

            def emit_warm(nmm):
                wp = pp.tile([128, 128], F32, tag="sel", name="warmps")
                for i in range(nmm):
                    nc.tensor.matmul(wp[:], warm_a[:], warm_b[:],
                                     start=(i == 0), stop=(i == nmm - 1))
                nc.scalar.activation(dbg_t[0:1, 15:16], wp[0:1, 0:1], AF.Copy)

            l_vals = []
            for si in range(BS):
                # ---- stream x: exact pass-through + exact GAP ----
                xus = []
                for kb in range(2):
                    xu = pw.tile([128, HW], F32, tag="xu", bufs=4)
                    nc.sync.dma_start(out=xu[:],
                                      in_=xp[si, kb * 128:(kb + 1) * 128]
                                      .rearrange("p a b -> p (a b)"))
                    xus.append(xu)
                if si == 0:
                    nc.sync.dma_start(out=wnat_t[:, 0:9 * C], in_=wnat[0])
                    nc.sync.dma_start(out=wnat_t[:, 9 * C:2 * 9 * C], in_=wnat[1])
                    nc.sync.dma_start(out=cgw_t[:, 0:9 * C], in_=cgw[0])
                    nc.sync.dma_start(out=cgw_t[:, 9 * C:2 * 9 * C], in_=cgw[1])
                    nc.sync.dma_start(out=fcwt_t[:, 0:C], in_=fcwt[0])
                    nc.sync.dma_start(out=fcwt_t[:, C:2 * C], in_=fcwt[1])
                    nc.sync.dma_start(out=cgb_t[:], in_=cgb[:])
                    nc.sync.dma_start(out=fcb_s[:], in_=fcb[:])
                    # mask threshold works on raw GAP sums: scale the bias by
                    # the 27x27 GAP count instead of dividing the sums
                    nc.vector.tensor_scalar_mul(fcb_s[:], fcb_s[:], 729.0)
                emit_warm(2)
                for kb in range(2):
                    col = kb * BS + si
                    if kb == 0:
                        nc.vector.tensor_reduce(
                            out=g1[:, col:col + 1], in_=xus[kb][:],
                            axis=mybir.AxisListType.X, op=ALU.add)
                    else:
                        gsc = pw.tile([128, HW], F32, tag="gapscr")
                        nc.scalar.activation(gsc[:], xus[kb][:], AF.Copy,
                                             accum_out=g1[:, col:col + 1])
                    nc.sync.dma_start(out=outp[si, kb * 128:(kb + 1) * 128],
                                      in_=xus[kb][:])

                # ---- layer gate (true fp32) for this sample ----
                ph = pp.tile([LSTM_H, 1], F32, tag="sel")
                for kb in range(2):
                    nc.tensor.matmul(
                        ph[:], lgwt_t[:, kb * LSTM_H:(kb + 1) * LSTM_H],
                        g1[:, kb * BS + si:kb * BS + si + 1],
                        start=(kb == 0), stop=(kb == 1))
                nc.scalar.activation(htile[0:LSTM_H, si:si + 1], ph[:], AF.Relu,
                                     bias=lgb_t[:, 0:1], scale=1.0 / HW)
                pg = pp.tile([1, 4 * LSTM_H], F32, tag="sel")
                nc.tensor.matmul(pg[:], htile[:, si:si + 1], wiht_t[:],
                                 start=True, stop=True)
                lw = pw.tile([1, 4 * LSTM_H], F32, tag="lw", bufs=2)
                nc.scalar.activation(lw[:, 0:LSTM_H], pg[:, 0:LSTM_H], AF.Sigmoid)
                nc.scalar.activation(lw[:, 3 * LSTM_H:4 * LSTM_H],
                                     pg[:, 3 * LSTM_H:4 * LSTM_H], AF.Sigmoid)
                nc.scalar.activation(lw[:, 2 * LSTM_H:3 * LSTM_H],
                                     pg[:, 2 * LSTM_H:3 * LSTM_H], AF.Tanh)
                cb_t = pw.tile([1, LSTM_H], F32, tag="cbuf", bufs=2)
                nc.vector.tensor_tensor(out=cb_t[:], in0=lw[:, 0:LSTM_H],
                                        in1=lw[:, 2 * LSTM_H:3 * LSTM_H],
                                        op=ALU.mult)
                eb_t = pw.tile([1, LSTM_H], F32, tag="ebuf", bufs=2)
                nc.scalar.activation(eb_t[:], cb_t[:], AF.Tanh)
                hs_t = pw.tile([1, LSTM_H], F32, tag="hsb", bufs=2)
                nc.vector.tensor_tensor(out=hs_t[:],
                                        in0=lw[:, 3 * LSTM_H:4 * LSTM_H],
                                        in1=eb_t[:], op=ALU.mult)
                pr_t = pw.tile([1, LSTM_H], F32, tag="prod", bufs=2)
                nc.vector.tensor_tensor(out=pr_t[:], in0=hs_t[:], in1=lgfc_t[:],
                                        op=ALU.mult)
                lpre = pw.tile([1, 1], F32, tag="lpre", bufs=2)
                nc.vector.tensor_reduce(out=lpre[:], in_=pr_t[:],
                                        axis=mybir.AxisListType.X, op=ALU.add)
                l_bin = pw.tile([1, 1], F32, tag="lbin", bufs=2)
                nc.vector.scalar_tensor_tensor(out=l_bin[:], in0=lpre[:],
                                               scalar=lfb_t[:, 0:1],
                                               in1=zeros1[:],
                                               op0=ALU.add, op1=ALU.is_gt)
                nc.vector.tensor_copy(out=dbg_t[0:1, si:si + 1], in_=lpre[:])
                nc.vector.tensor_copy(out=dbg_t[0:1, 4 + si:5 + si], in_=l_bin[:])
                l_i32 = pw.tile([1, 1], I32, tag="li32", bufs=2)
                nc.vector.tensor_copy(out=l_i32[:], in_=l_bin[:])
                l_vals.append(nc.values_load(l_i32[0:1, 0:1], engines=ENGINES,
                                             min_val=0, max_val=1,
                                             skip_runtime_bounds_check=True))

                # ---- gated heavy path for this sample ----
                with tc.For_i(0, l_vals[si], 1):
                    # re-load x as f32r: flat tiles for the stride-2 gate conv,
                    # zero-padded images for the main conv
                    xrs, xts = [], []
                    for kb in range(2):
                        xr = pw.tile([128, HW + 4], F32R, tag=f"xr{kb}")
                        nc.gpsimd.dma_start(out=xr[:, 0:HW],
                                            in_=xp[si, kb * 128:(kb + 1) * 128]
                                            .rearrange("p a b -> p (a b)"))
                        nc.vector.memset(xr[:, HW:HW + 4].bitcast(F32), 0.0)
                        xrs.append(xr)
                    for kb in range(2):
                        xt = pw.tile([128, XT_COLS], F32R, tag=f"xpad{kb}")
                        xv = xt[:, 0:PHW].rearrange("p (h w) -> p h w", h=PH)
                        nc.vector.memset(xv[:, 0:1, :].bitcast(F32), 0.0)
                        nc.vector.memset(xv[:, PH - 1:PH, :].bitcast(F32), 0.0)
                        nc.vector.memset(xv[:, :, 0:1].bitcast(F32), 0.0)
                        nc.vector.memset(xv[:, :, PW - 1:PW].bitcast(F32), 0.0)
                        nc.vector.memset(xt[:, PHW:XT_COLS].bitcast(F32), 0.0)
                        src = xrs[kb][:, 0:HW].bitcast(F32).rearrange(
                            "p (a b) -> p a b", a=H)
                        if kb == 0:
                            nc.vector.tensor_copy(
                                out=xv[:, 1:PH - 1, 1:PW - 1], in_=src)
                        else:
                            nc.scalar.activation(
                                xv[:, 1:PH - 1, 1:PW - 1], src, AF.Copy)
                        xts.append(xt)

                    # -- channel-gate conv (stride-2 valid 3x3) + GAP --
                    g2 = pw.tile([128, 2], F32, tag="g2")
                    for cb in range(2):
                        accs = []
                        for (r0, r1) in G_CHUNKS:
                            rows = r1 - r0
                            pgc = pp.tile([128, rows * G_COLS], F32, tag="conv",
                                          bufs=7)
                            first = True
                            for tap in range(9):
                                dy, dx = tap // 3, tap % 3
                                for kb in range(2):
                                    off = (2 * r0 + dy) * W + dx
                                    rhs = xrs[kb][:, off:off + 112 * rows] \
                                        .rearrange("p (a b) -> p a b", b=112) \
                                        [:, :, 0:2 * G_COLS:2]
                                    nc.tensor.matmul(
                                        pgc[:],
                                        cgw_t[:, kb * 9 * C + tap * C + cb * 128:
                                              kb * 9 * C + tap * C + cb * 128 + 128],
                                        rhs,
                                        start=first, stop=(tap == 8 and kb == 1))
                                    first = False
                            scr = pw.tile([128, 14 * G_COLS], F32, tag="gscr",
                                          bufs=2)
                            acc = pw.tile([128, 1], F32, tag=f"gacc{len(accs)}")
                            pv = pgc[:].rearrange("p (r c) -> p r c", c=G_COLS)
                            sv = scr[:].rearrange("p (r c) -> p r c", c=G_COLS)
                            nc.scalar.activation(sv[:, 0:rows, 0:27],
                                                 pv[:, :, 0:27], AF.Relu,
                                                 bias=cgb_t[:, cb:cb + 1],
                                                 accum_out=acc[:])
                            accs.append(acc)
                        nc.vector.tensor_tensor(out=g2[:, cb:cb + 1],
                                                in0=accs[0][:], in1=accs[1][:],
                                                op=ALU.add)

                    # -- fc -> binary mask per channel --
                    m_t = pw.tile([128, 2], F32, tag="mt")
                    for cbm in range(2):
                        pf = pp.tile([128, 1], F32, tag="sel")
                        for kb in range(2):
                            nc.tensor.matmul(
                                pf[:],
                                fcwt_t[:, kb * C + cbm * 128:kb * C + cbm * 128 + 128],
                                g2[:, kb:kb + 1],
                                start=(kb == 0), stop=(kb == 1))
                        nc.vector.scalar_tensor_tensor(
                            out=m_t[:, cbm:cbm + 1], in0=pf[:],
                            scalar=fcb_s[:, cbm:cbm + 1], in1=zeros128[:],
                            op0=ALU.add, op1=ALU.is_gt)

                    # -- n = #active, exclusive prefix, one-hot S, row indices --
                    pn = pp.tile([1, 1], F32, tag="sel")
                    for cb in range(2):
                        nc.tensor.matmul(pn[:], ones_t[:, 0:1], m_t[:, cb:cb + 1],
                                         start=(cb == 0), stop=(cb == 1))
                    n_sb = pw.tile([1, 1], F32, tag="nsb")
                    nc.scalar.activation(n_sb[:], pn[:], AF.Copy)
                    n_i32 = pw.tile([1, 2], I32, tag="ni32")
                    nc.vector.tensor_copy(out=n_i32[:, 0:1], in_=n_sb[:])
                    n2_sb = pw.tile([1, 1], F32, tag="n2sb")
                    nc.vector.tensor_scalar_sub(n2_sb[:], n_sb[:], 128.0)
                    nc.vector.tensor_copy(out=n_i32[:, 1:2], in_=n2_sb[:])
                    n_bc = pw.tile([128, 1], F32, tag="nbc")
                    nc.gpsimd.partition_broadcast(n_bc[:], n_sb[:])

                    pos_sb = pw.tile([128, 2], F32, tag="pos")
                    pp0 = pp.tile([128, 1], F32, tag="sel")
                    nc.tensor.matmul(pp0[:], ucon_t[:], m_t[:, 0:1],
                                     start=True, stop=True)
                    nc.scalar.activation(pos_sb[:, 0:1], pp0[:], AF.Copy)
                    pp1 = pp.tile([128, 1], F32, tag="sel")
                    nc.tensor.matmul(pp1[:], ones_t[:], m_t[:, 0:1],
                                     start=True, stop=False)
                    nc.tensor.matmul(pp1[:], ucon_t[:], m_t[:, 1:2],
                                     start=False, stop=True)
                    nc.scalar.activation(pos_sb[:, 1:2], pp1[:], AF.Copy)

                    s_ts = []
                    for cb in range(2):
                        s_t = pw.tile([128, 256], F32, tag=f"s{cb}")
                        nc.vector.tensor_scalar(
                            out=s_t[:].bitcast(F32R), in0=j_t[:],
                            scalar1=pos_sb[:, cb:cb + 1],
                            scalar2=None, op0=ALU.is_equal)
                        nc.vector.tensor_scalar(
                            out=s_t[:].bitcast(F32R), in0=s_t[:],
                            scalar1=m_t[:, cb:cb + 1], scalar2=None, op0=ALU.mult)
                        s_ts.append(s_t)

                    idx_i32 = pw.tile([128, 2], I32, tag="idxi")
                    for j in range(2):
                        pi = pp.tile([128, 1], F32, tag="sel")
                        for cb in range(2):
                            nc.tensor.matmul(pi[:],
                                             s_ts[cb][:, j * 128:(j + 1) * 128],
                                             cvec_t[:, cb:cb + 1],
                                             start=(cb == 0), stop=(cb == 1))
                        cmp = pw.tile([128, 1], F32, tag="cmp")
                        nc.vector.tensor_scalar(
                            out=cmp[:], in0=cvec_t[:, j:j + 1],
                            scalar1=n_bc[:, 0:1], scalar2=None, op0=ALU.is_ge)
                        idxf = pw.tile([128, 1], F32, tag="idxf")
                        nc.vector.scalar_tensor_tensor(
                            out=idxf[:], in0=cmp[:], scalar=4096.0, in1=pi[:],
                            op0=ALU.mult, op1=ALU.add)
                        nc.vector.tensor_scalar(
                            out=idxf[:], in0=idxf[:], scalar1=float(si * C),
                            scalar2=None, op0=ALU.add)
                        nc.vector.tensor_copy(out=idx_i32[:, j:j + 1], in_=idxf[:])

                    n_val = nc.values_load(n_i32[0:1, 0:1], engines=ENGINES,
                                           min_val=0, max_val=256,
                                           skip_runtime_bounds_check=True)
                    n2_val = nc.values_load(n_i32[0:1, 1:2], engines=ENGINES,
                                            min_val=-256, max_val=128,
                                            skip_runtime_bounds_check=True)

                    # -- weight gather interleaved with block-0 compact conv --
                    # selw[:, wi*256 + p] = W[idx_p, cin] for both p-blocks
                    selw = pw.tile([128, 18 * 256], F32R, tag="selw")
                    banks = [pp.tile([128, CH_N], F32, tag="conv", bufs=7,
                                     name=f"bank{_k}")
                             for _k in range(NCHUNK)]

                    def emit_sel(wi):
                        tap, kb = wi // 2, wi % 2
                        ps = pp.tile([128, 256], F32, tag="sel", name=f"ps{wi}")
                        for cb in range(2):
                            nc.tensor.matmul(
                                ps[:],
                                wnat_t[:, cb * 9 * C + tap * C + kb * 128:
                                       cb * 9 * C + tap * C + kb * 128 + 128],
                                s_ts[cb][:].bitcast(F32R),
                                start=(cb == 0), stop=(cb == 1))
                        nc.vector.tensor_copy(
                            out=selw[:, wi * 256:(wi + 1) * 256], in_=ps[:])

                    def emit_conv(wi, j):
                        tap, kb = wi // 2, wi % 2
                        dy, dx = tap // 3, tap % 3
                        xt = xts[kb]
                        for k in range(NCHUNK):
                            off = (CH_ROWS * k + dy) * PH + dx
                            nc.tensor.matmul(
                                banks[k][:],
                                selw[:, wi * 256 + j * 128:
                                     wi * 256 + j * 128 + 128],
                                xt[:, off:off + CH_N],
                                start=(wi == 0), stop=(wi == 17))

                    def emit_out(j):
                        stg = pw.tile([128, HW], F32, tag="stg", name=f"stg{j}")
                        for k in range(NCHUNK):
                            bv = banks[k][:].rearrange("p (r c) -> p r c", c=PH)
                            sv = stg[:].rearrange("p (r c) -> p r c", c=W)
                            nc.scalar.activation(
                                sv[:, k * CH_ROWS:(k + 1) * CH_ROWS, :],
                                bv[:, :, 0:W], AF.Copy)
                        nc.gpsimd.indirect_dma_start(
                            out=out_rows,
                            out_offset=bass.IndirectOffsetOnAxis(
                                ap=idx_i32[:, j:j + 1], axis=0),
                            in_=stg[:], in_offset=None,
                            bounds_check=BS * C - 1, oob_is_err=False)

                    emit_sel(0)
                    for wi in range(18):
                        if wi < 17:
                            emit_sel(wi + 1)
                        emit_conv(wi, 0)
                    emit_out(0)
                    with tc.For_i(0, smin(n2_val, 1), 1):
                        banks = [pp.tile([128, CH_N], F32, tag="conv", bufs=7,
                                         name=f"bankb{_k}")
                                 for _k in range(NCHUNK)]
                        for wi in range(18):
                            emit_conv(wi, 1)
                        emit_out(1)

            nc.sync.dma_start(out=dbg[:], in_=dbg_t[:])

    nc.compile()
    return nc


def _host_layouts(inputs):
    conv_w = np.asarray(inputs["conv_w"], np.float32)
    cg_conv_w = np.asarray(inputs["cg_conv_w"], np.float32)
    cg_fc_w = np.asarray(inputs["cg_fc_w"], np.float32)
    lg_conv_w = np.asarray(inputs["lg_conv_w"], np.float32)
    w_ih = np.asarray(inputs["lstm_w_ih"], np.float32)

    # wnat[cb][cout, tap*256+cin] = conv_w[cb*128+cout, cin, dy, dx]
    wn = conv_w.transpose(0, 2, 3, 1).reshape(C, 9 * C)
    wnat = np.ascontiguousarray(wn.reshape(2, 128, 9 * C))
    # cgw[kb][cin, tap*256+cout] = cg_conv_w[cout, kb*128+cin, dy, dx]
    cg = cg_conv_w.transpose(1, 2, 3, 0).reshape(C, 9 * C)
    cgw = np.ascontiguousarray(cg.reshape(2, 128, 9 * C))
    # fcwt[kb][k, c] = cg_fc_w[c, kb*128+k]
    fcwt = np.ascontiguousarray(cg_fc_w.T.reshape(2, 128, C))
    # lgwt[kb][k, m] = lg_conv_w[m, kb*128+k]
    lgwt = np.ascontiguousarray(
        lg_conv_w.reshape(LSTM_H, C).T.reshape(2, 128, LSTM_H))
    wiht = np.concatenate(
        [w_ih.T, (np.asarray(inputs["lstm_b_ih"], np.float32)
                  + np.asarray(inputs["lstm_b_hh"], np.float32))[None, :]],
        axis=0)
    wiht = np.ascontiguousarray(wiht)

    cgb = np.ascontiguousarray(
        np.asarray(inputs["cg_conv_b"], np.float32).reshape(2, 128).T)
    fcb = np.ascontiguousarray(
        np.asarray(inputs["cg_fc_b"], np.float32).reshape(2, 128).T)

    u = np.triu(np.ones((128, 128), np.float32), k=1)
    jc = np.tile(np.arange(256, dtype=np.float32)[None, :], (128, 1))
    cv = np.stack([np.arange(128, dtype=np.float32),
                   np.arange(128, 256, dtype=np.float32)], axis=1)

    return {
        "wnat": wnat, "cgw": cgw, "fcwt": fcwt, "lgwt": lgwt, "wiht": wiht,
        "lgfc": np.ascontiguousarray(
            np.asarray(inputs["lg_fc_w"], np.float32).reshape(1, LSTM_H)),
        "cgb": cgb, "fcb": fcb,
        "lgb": np.ascontiguousarray(
            np.asarray(inputs["lg_conv_b"], np.float32).reshape(LSTM_H, 1)),
        "lfb": np.ascontiguousarray(
            np.asarray(inputs["lg_fc_b"], np.float32).reshape(1, 1)),
        "ucon": np.ascontiguousarray(u),
        "onesk": np.ones((128, 128), np.float32),
        "jcon": np.ascontiguousarray(jc),
        "cvec": np.ascontiguousarray(cv),
    }


def kernel(**inputs):
    if "nc" not in _CACHE:
        _CACHE["nc"] = _build()
    nc = _CACHE["nc"]

    x = np.asarray(inputs["x"], np.float32)
    shared = _host_layouts(inputs)
    in_maps = []
    for core in range(NCORES):
        m = dict(shared)
        m["x"] = np.ascontiguousarray(x[core * BS:(core + 1) * BS])
        in_maps.append(m)

    trace = bool(int(os.environ.get("BASS_KERNEL_TRACE", "0")))
    kw = {}
    if trace:
        from trn_agent_boot.trn_boot import _ntff_profile_via_ctypes
        import antenv.axon_hooks as ah
        ah.set_axon_ntff_profile_hook(
            _ntff_profile_via_ctypes("/opt/axon/libaxon_pjrt.so"))
        import tempfile
        base = os.environ.get("BASS_KERNEL_TRACE_DIR", "/tmp/adaptconv_trace")
        os.makedirs(base, exist_ok=True)
        kw = dict(trace=True, tmpdir=tempfile.mkdtemp(dir=base))

    res = run_bass_kernel_spmd(nc, in_maps, core_ids=list(range(NCORES)), **kw)
    _CACHE["last_exec_time_ns"] = res.exec_time_ns

    _CACHE["dbg"] = [res.results[i].get("dbg") for i in range(NCORES)]
    out = np.concatenate(
        [res.results[i]["out"].reshape(BS, C, H, W) for i in range(NCORES)],
        axis=0)
    return out


# revision 22
# speedup vs baseline: 1.4902x; 1.0109x over previous
"""AdaptConv2d Trainium2 kernel: 8-core data-parallel, gate-driven sparse conv.

Computes, per sample b:
  layer_bit = (LSTM-gate pre-activation > 0)
  if layer_bit:  channel mask m_c = (channel-gate fc pre-activation > 0)
                 out[c] = conv3x3(x)[c] if m_c else x[c]
  else:          out = x

Device strategy per core (4 samples):
  - x loaded into zero-padded (58x58) SBUF images, f32r (tf32) typed for the
    TensorEngine fast path; bits are untouched so pass-through output is exact.
  - Layer gate (GAP + 1x1-conv + single-step LSTM + fc) in true fp32 for all 4
    samples, branch-free.
  - Per sample, a 0/1-trip For_i (trip = layer bit) guards the heavy work:
    stride-2 channel-gate conv (tf32 matmuls, fp32 accum), fp32 fc -> binary
    mask, prefix-sum -> one-hot selection matrix S, PE-side weight gather
    (W^T @ S), compact conv over only ceil(n_active/128) 128-channel blocks
    (inner 0/1-trip For_i for the second block), and an indirect row-scatter
    of conv rows into the output (out-of-bounds pad rows silently dropped).
  - Unconditional default write out = x covers inactive channels/samples.
"""

import os
import sys
import types

sys.path.insert(0, "/opt/trn_rl_repo")

import numpy as np

# antenv.axon_hooks is missing from this image; inject a minimal stand-in so
# run_bass_kernel_spmd's trace path imports cleanly (used only when tracing).
try:
    import antenv  # noqa: F401

    if "antenv.axon_hooks" not in sys.modules:
        _m = types.ModuleType("antenv.axon_hooks")
        _h = [None]
        _m.set_axon_ntff_profile_hook = lambda hook: _h.__setitem__(0, hook)
        _m.get_axon_ntff_profile_hook = lambda: _h[0]
        sys.modules["antenv.axon_hooks"] = _m
        antenv.axon_hooks = _m
except Exception:
    pass

import concourse.bass as bass
import concourse.mybir as mybir
from concourse import bacc
from concourse.expressions import smin
from concourse.tile import TileContext
from concourse.bass_utils import run_bass_kernel_spmd

F32 = mybir.dt.float32
F32R = mybir.dt.float32r
I32 = mybir.dt.int32
AF = mybir.ActivationFunctionType
ALU = mybir.AluOpType

B, C, H, W = 32, 256, 56, 56
NCORES = 8
BS = B // NCORES          # samples per core
HW = H * W                # 3136
PH, PW = H + 2, W + 2     # 58x58 padded image
PHW = PH * PW             # 3364
XT_COLS = PHW + 4         # tail pad: edge-tap reads run 2 past the image
LSTM_H = 10
ENGINES = list(mybir.ALL_ENGINES)

# main-conv spatial chunking: 7 chunks x 8 valid rows; each chunk is a
# contiguous 464-wide span of the padded image (includes L/R pad cols, whose
# outputs are junk and excluded at extraction time)
NCHUNK = 7
CH_ROWS = 8
CH_N = CH_ROWS * PH       # 464

# channel-gate conv: 27x27 valid outputs, row-chunks of 14/13, 28 cols (28th
# col junk so the fp32r moving operand has an even innermost count)
G_CHUNKS = ((0, 14), (14, 27))
G_COLS = 28

_CACHE = {}


def _build():
    nc = bacc.Bacc(None, target_bir_lowering=False)

    xp = nc.declare_dram_parameter("x", [BS, C, H, W], F32, isOutput=False)
    outp = nc.declare_dram_parameter("out", [BS, C, HW], F32, isOutput=True)
    wnat = nc.declare_dram_parameter("wnat", [2, 128, 9 * C], F32R, isOutput=False)
    cgw = nc.declare_dram_parameter("cgw", [2, 128, 9 * C], F32R, isOutput=False)
    fcwt = nc.declare_dram_parameter("fcwt", [2, 128, C], F32, isOutput=False)
    lgwt = nc.declare_dram_parameter("lgwt", [2, 128, LSTM_H], F32, isOutput=False)
    wiht = nc.declare_dram_parameter("wiht", [LSTM_H + 1, 4 * LSTM_H], F32, isOutput=False)
    lgfc = nc.declare_dram_parameter("lgfc", [1, LSTM_H], F32, isOutput=False)
    cgb = nc.declare_dram_parameter("cgb", [128, 2], F32, isOutput=False)
    fcb = nc.declare_dram_parameter("fcb", [128, 2], F32, isOutput=False)
    lgb = nc.declare_dram_parameter("lgb", [LSTM_H, 1], F32, isOutput=False)
    lfb = nc.declare_dram_parameter("lfb", [1, 1], F32, isOutput=False)
    ucon = nc.declare_dram_parameter("ucon", [128, 128], F32, isOutput=False)
    onesk = nc.declare_dram_parameter("onesk", [128, 128], F32, isOutput=False)
    jcon = nc.declare_dram_parameter("jcon", [128, 2 * 128], F32, isOutput=False)
    cvec = nc.declare_dram_parameter("cvec", [128, 2], F32, isOutput=False)
    dbg = nc.declare_dram_parameter("dbg", [128, 16], F32, isOutput=True)

    with TileContext(nc) as tc:
        with tc.tile_pool(name="sbuf", bufs=1) as pc, \
             tc.tile_pool(name="work", bufs=1) as pw, \
             tc.tile_pool(name="psum", bufs=1, space="PSUM") as pp:

            # ---- constants / weights resident in SBUF ----
            ucon_t = pc.tile([128, 128], F32, tag="ucon")
            nc.sync.dma_start(out=ucon_t[:], in_=ucon[:])
            ones_t = pc.tile([128, 128], F32, tag="ones")
            nc.sync.dma_start(out=ones_t[:], in_=onesk[:])
            j_t = pc.tile([128, 256], F32, tag="jcon")
            nc.sync.dma_start(out=j_t[:], in_=jcon[:])
            cvec_t = pc.tile([128, 2], F32, tag="cvec")
            nc.sync.dma_start(out=cvec_t[:], in_=cvec[:])
            lgwt_t = pc.tile([128, 2 * LSTM_H], F32, tag="lgwt")
            nc.sync.dma_start(out=lgwt_t[:, 0:LSTM_H], in_=lgwt[0])
            nc.sync.dma_start(out=lgwt_t[:, LSTM_H:2 * LSTM_H], in_=lgwt[1])
            wiht_t = pc.tile([LSTM_H + 1, 4 * LSTM_H], F32, tag="wiht")
            nc.sync.dma_start(out=wiht_t[:], in_=wiht[:])
            lgb_t = pc.tile([LSTM_H, 1], F32, tag="lgb")
            nc.sync.dma_start(out=lgb_t[:], in_=lgb[:])
            lgfc_t = pc.tile([1, LSTM_H], F32, tag="lgfc")
            nc.sync.dma_start(out=lgfc_t[:], in_=lgfc[:])
            lfb_t = pc.tile([1, 1], F32, tag="lfb")
            nc.sync.dma_start(out=lfb_t[:], in_=lfb[:])

            zeros1 = pc.tile([1, 1], F32, tag="z1")
            nc.vector.memset(zeros1[:], 0.0)
            zeros128 = pc.tile([128, 1], F32, tag="z128")
            nc.vector.memset(zeros128[:], 0.0)

            out_rows = outp[:].rearrange("a c n -> (a c) n")
            g1 = pc.tile([128, 2 * BS], F32, tag="g1")   # GAP sums, col kb*BS+si
            htile = pc.tile([LSTM_H + 1, BS], F32, tag="htile")
            nc.sync.dma_start(out=htile[LSTM_H:LSTM_H + 1, :],
                              in_=onesk[0:1, 0:BS])
            dbg_t = pc.tile([128, 16], F32, tag="dbg")
            nc.vector.memset(dbg_t[:], 0.0)

            # conv/gate weights (stream in behind the first x tiles)
            wnat_t = pc.tile([128, 2 * 9 * C], F32R, tag="wnat")
            cgw_t = pc.tile([128, 2 * 9 * C], F32R, tag="cgw")
            fcwt_t = pc.tile([128, 2 * C], F32, tag="fcwt")
            cgb_t = pc.tile([128, 2], F32, tag="cgb")
            fcb_s = pc.tile([128, 2], F32, tag="fcbs")

            warm_a = pc.tile([128, 128], F32, tag="warma")
            nc.sync.dma_start(out=warm_a[:], in_=onesk[:])
            warm_b = pc.tile([128, 128], F32, tag="warmb")
            nc.sync.dma_start(out=warm_b[:], in_=onesk[:])

            def emit_warm(nmm):
                wp = pp.tile([128, 128], F32, tag="sel", name="warmps")
                for i in range(nmm):
                    nc.tensor.matmul(wp[:], warm_a[:], warm_b[:],
                                     start=True, stop=True,
                                     skip_group_check=True)
                nc.scalar.activation(dbg_t[0:1, 15:16], wp[0:1, 0:1], AF.Copy)

            l_vals = []
            for si in range(BS):
                # ---- stream x: exact pass-through + exact GAP ----
                xus = []
                for kb in range(2):
                    xu = pw.tile([128, HW], F32, tag="xu", bufs=4)
                    nc.sync.dma_start(out=xu[:],
                                      in_=xp[si, kb * 128:(kb + 1) * 128]
                                      .rearrange("p a b -> p (a b)"))
                    xus.append(xu)
                if si == 0:
                    nc.sync.dma_start(out=wnat_t[:, 0:9 * C], in_=wnat[0])
                    nc.sync.dma_start(out=wnat_t[:, 9 * C:2 * 9 * C], in_=wnat[1])
                    nc.sync.dma_start(out=cgw_t[:, 0:9 * C], in_=cgw[0])
                    nc.sync.dma_start(out=cgw_t[:, 9 * C:2 * 9 * C], in_=cgw[1])
                    nc.sync.dma_start(out=fcwt_t[:, 0:C], in_=fcwt[0])
                    nc.sync.dma_start(out=fcwt_t[:, C:2 * C], in_=fcwt[1])
                    nc.sync.dma_start(out=cgb_t[:], in_=cgb[:])
                    nc.sync.dma_start(out=fcb_s[:], in_=fcb[:])
                    # mask threshold works on raw GAP sums: scale the bias by
                    # the 27x27 GAP count instead of dividing the sums
                    nc.vector.tensor_scalar_mul(fcb_s[:], fcb_s[:], 729.0)
                emit_warm(12)
                for kb in range(2):
                    col = kb * BS + si
                    if kb == 0:
                        nc.vector.tensor_reduce(
                            out=g1[:, col:col + 1], in_=xus[kb][:],
                            axis=mybir.AxisListType.X, op=ALU.add)
                    else:
                        gsc = pw.tile([128, HW], F32, tag="gapscr")
                        nc.scalar.activation(gsc[:], xus[kb][:], AF.Copy,
                                             accum_out=g1[:, col:col + 1])
                    nc.sync.dma_start(out=outp[si, kb * 128:(kb + 1) * 128],
                                      in_=xus[kb][:])

                # ---- layer gate (true fp32) for this sample ----
                ph = pp.tile([LSTM_H, 1], F32, tag="sel")
                for kb in range(2):
                    nc.tensor.matmul(
                        ph[:], lgwt_t[:, kb * LSTM_H:(kb + 1) * LSTM_H],
                        g1[:, kb * BS + si:kb * BS + si + 1],
                        start=(kb == 0), stop=(kb == 1))
                nc.scalar.activation(htile[0:LSTM_H, si:si + 1], ph[:], AF.Relu,
                                     bias=lgb_t[:, 0:1], scale=1.0 / HW)
                pg = pp.tile([1, 4 * LSTM_H], F32, tag="sel")
                nc.tensor.matmul(pg[:], htile[:, si:si + 1], wiht_t[:],
                                 start=True, stop=True)
                lw = pw.tile([1, 4 * LSTM_H], F32, tag="lw", bufs=2)
                nc.scalar.activation(lw[:, 0:LSTM_H], pg[:, 0:LSTM_H], AF.Sigmoid)
                nc.scalar.activation(lw[:, 3 * LSTM_H:4 * LSTM_H],
                                     pg[:, 3 * LSTM_H:4 * LSTM_H], AF.Sigmoid)
                nc.scalar.activation(lw[:, 2 * LSTM_H:3 * LSTM_H],
                                     pg[:, 2 * LSTM_H:3 * LSTM_H], AF.Tanh)
                cb_t = pw.tile([1, LSTM_H], F32, tag="cbuf", bufs=2)
                nc.vector.tensor_tensor(out=cb_t[:], in0=lw[:, 0:LSTM_H],
                                        in1=lw[:, 2 * LSTM_H:3 * LSTM_H],
                                        op=ALU.mult)
                eb_t = pw.tile([1, LSTM_H], F32, tag="ebuf", bufs=2)
                nc.scalar.activation(eb_t[:], cb_t[:], AF.Tanh)
                hs_t = pw.tile([1, LSTM_H], F32, tag="hsb", bufs=2)
                nc.vector.tensor_tensor(out=hs_t[:],
                                        in0=lw[:, 3 * LSTM_H:4 * LSTM_H],
                                        in1=eb_t[:], op=ALU.mult)
                pr_t = pw.tile([1, LSTM_H], F32, tag="prod", bufs=2)
                nc.vector.tensor_tensor(out=pr_t[:], in0=hs_t[:], in1=lgfc_t[:],
                                        op=ALU.mult)
                lpre = pw.tile([1, 1], F32, tag="lpre", bufs=2)
                nc.vector.tensor_reduce(out=lpre[:], in_=pr_t[:],
                                        axis=mybir.AxisListType.X, op=ALU.add)
                l_bin = pw.tile([1, 1], F32, tag="lbin", bufs=2)
                nc.vector.scalar_tensor_tensor(out=l_bin[:], in0=lpre[:],
                                               scalar=lfb_t[:, 0:1],
                                               in1=zeros1[:],
                                               op0=ALU.add, op1=ALU.is_gt)
                nc.vector.tensor_copy(out=dbg_t[0:1, si:si + 1], in_=lpre[:])
                nc.vector.tensor_copy(out=dbg_t[0:1, 4 + si:5 + si], in_=l_bin[:])
                l_i32 = pw.tile([1, 1], I32, tag="li32", bufs=2)
                nc.vector.tensor_copy(out=l_i32[:], in_=l_bin[:])
                l_vals.append(nc.values_load(l_i32[0:1, 0:1], engines=ENGINES,
                                             min_val=0, max_val=1,
                                             skip_runtime_bounds_check=True))

                # ---- gated heavy path for this sample ----
                with tc.For_i(0, l_vals[si], 1):
                    # re-load x as f32r: flat tiles for the stride-2 gate conv,
                    # zero-padded images for the main conv
                    xrs, xts = [], []
                    for kb in range(2):
                        xr = pw.tile([128, HW + 4], F32R, tag=f"xr{kb}")
                        nc.gpsimd.dma_start(out=xr[:, 0:HW],
                                            in_=xp[si, kb * 128:(kb + 1) * 128]
                                            .rearrange("p a b -> p (a b)"))
                        nc.vector.memset(xr[:, HW:HW + 4].bitcast(F32), 0.0)
                        xrs.append(xr)
                    for kb in range(2):
                        xt = pw.tile([128, XT_COLS], F32R, tag=f"xpad{kb}")
                        xv = xt[:, 0:PHW].rearrange("p (h w) -> p h w", h=PH)
                        nc.vector.memset(xv[:, 0:1, :].bitcast(F32), 0.0)
                        nc.vector.memset(xv[:, PH - 1:PH, :].bitcast(F32), 0.0)
                        nc.vector.memset(xv[:, :, 0:1].bitcast(F32), 0.0)
                        nc.vector.memset(xv[:, :, PW - 1:PW].bitcast(F32), 0.0)
                        nc.vector.memset(xt[:, PHW:XT_COLS].bitcast(F32), 0.0)
                        src = xrs[kb][:, 0:HW].bitcast(F32).rearrange(
                            "p (a b) -> p a b", a=H)
                        if kb == 0:
                            nc.vector.tensor_copy(
                                out=xv[:, 1:PH - 1, 1:PW - 1], in_=src)
                        else:
                            nc.scalar.activation(
                                xv[:, 1:PH - 1, 1:PW - 1], src, AF.Copy)
                        xts.append(xt)

                    # -- channel-gate conv (stride-2 valid 3x3) + GAP --
                    g2 = pw.tile([128, 2], F32, tag="g2")
                    for cb in range(2):
                        accs = []
                        for (r0, r1) in G_CHUNKS:
                            rows = r1 - r0
                            pgc = pp.tile([128, rows * G_COLS], F32, tag="conv",
                                          bufs=7)
                            first = True
                            for tap in range(9):
                                dy, dx = tap // 3, tap % 3
                                for kb in range(2):
                                    off = (2 * r0 + dy) * W + dx
                                    rhs = xrs[kb][:, off:off + 112 * rows] \
                                        .rearrange("p (a b) -> p a b", b=112) \
                                        [:, :, 0:2 * G_COLS:2]
                                    nc.tensor.matmul(
                                        pgc[:],
                                        cgw_t[:, kb * 9 * C + tap * C + cb * 128:
                                              kb * 9 * C + tap * C + cb * 128 + 128],
                                        rhs,
                                        start=first, stop=(tap == 8 and kb == 1))
                                    first = False
                            scr = pw.tile([128, 14 * G_COLS], F32, tag="gscr",
                                          bufs=2)
                            acc = pw.tile([128, 1], F32, tag=f"gacc{len(accs)}")
                            pv = pgc[:].rearrange("p (r c) -> p r c", c=G_COLS)
                            sv = scr[:].rearrange("p (r c) -> p r c", c=G_COLS)
                            nc.scalar.activation(sv[:, 0:rows, 0:27],
                                                 pv[:, :, 0:27], AF.Relu,
                                                 bias=cgb_t[:, cb:cb + 1],
                                                 accum_out=acc[:])
                            accs.append(acc)
                        nc.vector.tensor_tensor(out=g2[:, cb:cb + 1],
                                                in0=accs[0][:], in1=accs[1][:],
                                                op=ALU.add)

                    # -- fc -> binary mask per channel --
                    m_t = pw.tile([128, 2], F32, tag="mt")
                    for cbm in range(2):
                        pf = pp.tile([128, 1], F32, tag="sel")
                        for kb in range(2):
                            nc.tensor.matmul(
                                pf[:],
                                fcwt_t[:, kb * C + cbm * 128:kb * C + cbm * 128 + 128],
                                g2[:, kb:kb + 1],
                                start=(kb == 0), stop=(kb == 1))
                        nc.vector.scalar_tensor_tensor(
                            out=m_t[:, cbm:cbm + 1], in0=pf[:],
                            scalar=fcb_s[:, cbm:cbm + 1], in1=zeros128[:],
                            op0=ALU.add, op1=ALU.is_gt)

                    # -- n = #active, exclusive prefix, one-hot S, row indices --
                    pn = pp.tile([1, 1], F32, tag="sel")
                    for cb in range(2):
                        nc.tensor.matmul(pn[:], ones_t[:, 0:1], m_t[:, cb:cb + 1],
                                         start=(cb == 0), stop=(cb == 1))
                    n_sb = pw.tile([1, 1], F32, tag="nsb")
                    nc.scalar.activation(n_sb[:], pn[:], AF.Copy)
                    n_i32 = pw.tile([1, 2], I32, tag="ni32")
                    nc.vector.tensor_copy(out=n_i32[:, 0:1], in_=n_sb[:])
                    n2_sb = pw.tile([1, 1], F32, tag="n2sb")
                    nc.vector.tensor_scalar_sub(n2_sb[:], n_sb[:], 128.0)
                    nc.vector.tensor_copy(out=n_i32[:, 1:2], in_=n2_sb[:])
                    n_bc = pw.tile([128, 1], F32, tag="nbc")
                    nc.gpsimd.partition_broadcast(n_bc[:], n_sb[:])

                    pos_sb = pw.tile([128, 2], F32, tag="pos")
                    pp0 = pp.tile([128, 1], F32, tag="sel")
                    nc.tensor.matmul(pp0[:], ucon_t[:], m_t[:, 0:1],
                                     start=True, stop=True)
                    nc.scalar.activation(pos_sb[:, 0:1], pp0[:], AF.Copy)
                    pp1 = pp.tile([128, 1], F32, tag="sel")
                    nc.tensor.matmul(pp1[:], ones_t[:], m_t[:, 0:1],
                                     start=True, stop=False)
                    nc.tensor.matmul(pp1[:], ucon_t[:], m_t[:, 1:2],
                                     start=False, stop=True)
                    nc.scalar.activation(pos_sb[:, 1:2], pp1[:], AF.Copy)

                    s_ts = []
                    for cb in range(2):
                        s_t = pw.tile([128, 256], F32, tag=f"s{cb}")
                        nc.vector.tensor_scalar(
                            out=s_t[:].bitcast(F32R), in0=j_t[:],
                            scalar1=pos_sb[:, cb:cb + 1],
                            scalar2=None, op0=ALU.is_equal)
                        nc.vector.tensor_scalar(
                            out=s_t[:].bitcast(F32R), in0=s_t[:],
                            scalar1=m_t[:, cb:cb + 1], scalar2=None, op0=ALU.mult)
                        s_ts.append(s_t)

                    idx_i32 = pw.tile([128, 2], I32, tag="idxi")
                    for j in range(2):
                        pi = pp.tile([128, 1], F32, tag="sel")
                        for cb in range(2):
                            nc.tensor.matmul(pi[:],
                                             s_ts[cb][:, j * 128:(j + 1) * 128],
                                             cvec_t[:, cb:cb + 1],
                                             start=(cb == 0), stop=(cb == 1))
                        cmp = pw.tile([128, 1], F32, tag="cmp")
                        nc.vector.tensor_scalar(
                            out=cmp[:], in0=cvec_t[:, j:j + 1],
                            scalar1=n_bc[:, 0:1], scalar2=None, op0=ALU.is_ge)
                        idxf = pw.tile([128, 1], F32, tag="idxf")
                        nc.vector.scalar_tensor_tensor(
                            out=idxf[:], in0=cmp[:], scalar=4096.0, in1=pi[:],
                            op0=ALU.mult, op1=ALU.add)
                        nc.vector.tensor_scalar(
                            out=idxf[:], in0=idxf[:], scalar1=float(si * C),
                            scalar2=None, op0=ALU.add)
                        nc.vector.tensor_copy(out=idx_i32[:, j:j + 1], in_=idxf[:])

                    n_val = nc.values_load(n_i32[0:1, 0:1], engines=ENGINES,
                                           min_val=0, max_val=256,
                                           skip_runtime_bounds_check=True)
                    n2_val = nc.values_load(n_i32[0:1, 1:2], engines=ENGINES,
                                            min_val=-256, max_val=128,
                                            skip_runtime_bounds_check=True)

                    # -- weight gather interleaved with block-0 compact conv --
                    # selw[:, wi*256 + p] = W[idx_p, cin] for both p-blocks
                    selw = pw.tile([128, 18 * 256], F32R, tag="selw")
                    banks = [pp.tile([128, CH_N], F32, tag="conv", bufs=7,
                                     name=f"bank{_k}")
                             for _k in range(NCHUNK)]

                    def emit_sel(wi):
                        tap, kb = wi // 2, wi % 2
                        ps = pp.tile([128, 256], F32, tag="sel", name=f"ps{wi}")
                        for cb in range(2):
                            nc.tensor.matmul(
                                ps[:],
                                wnat_t[:, cb * 9 * C + tap * C + kb * 128:
                                       cb * 9 * C + tap * C + kb * 128 + 128],
                                s_ts[cb][:].bitcast(F32R),
                                start=(cb == 0), stop=(cb == 1))
                        nc.vector.tensor_copy(
                            out=selw[:, wi * 256:(wi + 1) * 256], in_=ps[:])

                    def emit_conv(wi, j):
                        tap, kb = wi // 2, wi % 2
                        dy, dx = tap // 3, tap % 3
                        xt = xts[kb]
                        for k in range(NCHUNK):
                            off = (CH_ROWS * k + dy) * PH + dx
                            nc.tensor.matmul(
                                banks[k][:],
                                selw[:, wi * 256 + j * 128:
                                     wi * 256 + j * 128 + 128],
                                xt[:, off:off + CH_N],
                                start=(wi == 0), stop=(wi == 17))

                    def emit_out(j):
                        stg = pw.tile([128, HW], F32, tag="stg", name=f"stg{j}")
                        for k in range(NCHUNK):
                            bv = banks[k][:].rearrange("p (r c) -> p r c", c=PH)
                            sv = stg[:].rearrange("p (r c) -> p r c", c=W)
                            nc.scalar.activation(
                                sv[:, k * CH_ROWS:(k + 1) * CH_ROWS, :],
                                bv[:, :, 0:W], AF.Copy)
                        nc.gpsimd.indirect_dma_start(
                            out=out_rows,
                            out_offset=bass.IndirectOffsetOnAxis(
                                ap=idx_i32[:, j:j + 1], axis=0),
                            in_=stg[:], in_offset=None,
                            bounds_check=BS * C - 1, oob_is_err=False)

                    emit_sel(0)
                    for wi in range(18):
                        if wi < 17:
                            emit_sel(wi + 1)
                        emit_conv(wi, 0)
                    emit_out(0)
                    with tc.For_i(0, smin(n2_val, 1), 1):
                        banks = [pp.tile([128, CH_N], F32, tag="conv", bufs=7,
                                         name=f"bankb{_k}")
                                 for _k in range(NCHUNK)]
                        for wi in range(18):
                            emit_conv(wi, 1)
                        emit_out(1)

            nc.sync.dma_start(out=dbg[:], in_=dbg_t[:])

    nc.compile()
    return nc


def _host_layouts(inputs):
    conv_w = np.asarray(inputs["conv_w"], np.float32)
    cg_conv_w = np.asarray(inputs["cg_conv_w"], np.float32)
    cg_fc_w = np.asarray(inputs["cg_fc_w"], np.float32)
    lg_conv_w = np.asarray(inputs["lg_conv_w"], np.float32)
    w_ih = np.asarray(inputs["lstm_w_ih"], np.float32)

    # wnat[cb][cout, tap*256+cin] = conv_w[cb*128+cout, cin, dy, dx]
    wn = conv_w.transpose(0, 2, 3, 1).reshape(C, 9 * C)
    wnat = np.ascontiguousarray(wn.reshape(2, 128, 9 * C))
    # cgw[kb][cin, tap*256+cout] = cg_conv_w[cout, kb*128+cin, dy, dx]
    cg = cg_conv_w.transpose(1, 2, 3, 0).reshape(C, 9 * C)
    cgw = np.ascontiguousarray(cg.reshape(2, 128, 9 * C))
    # fcwt[kb][k, c] = cg_fc_w[c, kb*128+k]
    fcwt = np.ascontiguousarray(cg_fc_w.T.reshape(2, 128, C))
    # lgwt[kb][k, m] = lg_conv_w[m, kb*128+k]
    lgwt = np.ascontiguousarray(
        lg_conv_w.reshape(LSTM_H, C).T.reshape(2, 128, LSTM_H))
    wiht = np.concatenate(
        [w_ih.T, (np.asarray(inputs["lstm_b_ih"], np.float32)
                  + np.asarray(inputs["lstm_b_hh"], np.float32))[None, :]],
        axis=0)
    wiht = np.ascontiguousarray(wiht)

    cgb = np.ascontiguousarray(
        np.asarray(inputs["cg_conv_b"], np.float32).reshape(2, 128).T)
    fcb = np.ascontiguousarray(
        np.asarray(inputs["cg_fc_b"], np.float32).reshape(2, 128).T)

    u = np.triu(np.ones((128, 128), np.float32), k=1)
    jc = np.tile(np.arange(256, dtype=np.float32)[None, :], (128, 1))
    cv = np.stack([np.arange(128, dtype=np.float32),
                   np.arange(128, 256, dtype=np.float32)], axis=1)

    return {
        "wnat": wnat, "cgw": cgw, "fcwt": fcwt, "lgwt": lgwt, "wiht": wiht,
        "lgfc": np.ascontiguousarray(
            np.asarray(inputs["lg_fc_w"], np.float32).reshape(1, LSTM_H)),
        "cgb": cgb, "fcb": fcb,
        "lgb": np.ascontiguousarray(
            np.asarray(inputs["lg_conv_b"], np.float32).reshape(LSTM_H, 1)),
        "lfb": np.ascontiguousarray(
            np.asarray(inputs["lg_fc_b"], np.float32).reshape(1, 1)),
        "ucon": np.ascontiguousarray(u),
        "onesk": np.ones((128, 128), np.float32),
        "jcon": np.ascontiguousarray(jc),
        "cvec": np.ascontiguousarray(cv),
    }


def kernel(**inputs):
    if "nc" not in _CACHE:
        _CACHE["nc"] = _build()
    nc = _CACHE["nc"]

    x = np.asarray(inputs["x"], np.float32)
    shared = _host_layouts(inputs)
    in_maps = []
    for core in range(NCORES):
        m = dict(shared)
        m["x"] = np.ascontiguousarray(x[core * BS:(core + 1) * BS])
        in_maps.append(m)

    trace = bool(int(os.environ.get("BASS_KERNEL_TRACE", "0")))
    kw = {}
    if trace:
        from trn_agent_boot.trn_boot import _ntff_profile_via_ctypes
        import antenv.axon_hooks as ah
        ah.set_axon_ntff_profile_hook(
            _ntff_profile_via_ctypes("/opt/axon/libaxon_pjrt.so"))
        import tempfile
        base = os.environ.get("BASS_KERNEL_TRACE_DIR", "/tmp/adaptconv_trace")
        os.makedirs(base, exist_ok=True)
        kw = dict(trace=True, tmpdir=tempfile.mkdtemp(dir=base))

    res = run_bass_kernel_spmd(nc, in_maps, core_ids=list(range(NCORES)), **kw)
    _CACHE["last_exec_time_ns"] = res.exec_time_ns

    _CACHE["dbg"] = [res.results[i].get("dbg") for i in range(NCORES)]
    out = np.concatenate(
        [res.results[i]["out"].reshape(BS, C, H, W) for i in range(NCORES)],
        axis=0)
    return out
